# revision 24
# baseline (speedup 1.0000x reference)
"""Trainium2 Bass kernel for nn_KnowledgeBaseModule.

Data-parallel over batch: 8 batch rows -> 8 NeuronCores, weights/tables
replicated. Entity encoder (the dominant compute) runs in feature-major
(transposed) layout so both weight matmuls use natural weight layouts; the
two big matmuls use float32r (fast fp32) operands, everything index-critical
stays fp32.
"""

import numpy as np

import concourse.bass as bass
import concourse.mybir as mybir
import concourse.tile as tile
from concourse import bacc
from concourse.masks import make_identity

f32 = mybir.dt.float32
f32r = mybir.dt.float32r
i32 = mybir.dt.int32
u32 = mybir.dt.uint32

P = 128
H = 1024
H2 = 2048
T = 2048          # tokens per core (one batch row)
TB = 512          # token block
NBLK = T // TB    # 4
NE = 1000
TOPK = 5
EPS = 1e-5
KO1 = H // P      # 8
MO1 = H2 // P     # 16
KO2 = H2 // P     # 16
MO2 = H // P      # 8
N_CORES = 8

AF = mybir.ActivationFunctionType
ALU = mybir.AluOpType


def _row_to_col(nc, ps_pool, one1, col_pool, row, n_chunks, name):
    """[1, n_chunks*128] row -> [128, n_chunks] column layout via K=1 matmuls."""
    ps_col = ps_pool.tile([P, n_chunks], f32, name=f"{name}_ps", tag="colacc")
    for ko in range(n_chunks):
        nc.tensor.matmul(
            ps_col[:, ko : ko + 1],
            lhsT=row[:, ko * P : (ko + 1) * P],
            rhs=one1,
            start=True,
            stop=True,
        )
    col = col_pool.tile([P, n_chunks], f32, name=f"{name}_col", tag=f"{name}_col")
    nc.vector.tensor_copy(col, ps_col)
    return col


def _row_layernorm(nc, rows_pool, scal_pool, row, g_row, b_row, eps1, name):
    """LayerNorm along the free dim of a [1, N] row (single partition)."""
    n = row.shape[1]

    def scal(nm):
        return scal_pool.tile([1, 1], f32, name=f"{name}_{nm}", tag="scal")

    s1 = scal("s1")
    s2 = scal("s2")
    sq = rows_pool.tile([1, n], f32, name=f"{name}_sq", tag="rows", bufs=6)
    nc.vector.tensor_reduce(s1, row, axis=mybir.AxisListType.X, op=ALU.add)
    nc.vector.tensor_tensor(sq, row, row, ALU.mult)
    nc.vector.tensor_reduce(s2, sq, axis=mybir.AxisListType.X, op=ALU.add)
    mean = scal("mean")
    msq = scal("msq")
    nc.scalar.mul(mean, s1, 1.0 / n)
    nc.scalar.mul(msq, s2, 1.0 / n)
    m2 = scal("m2")
    var = scal("var")
    nc.vector.tensor_tensor(m2, mean, mean, ALU.mult)
    nc.vector.tensor_tensor(var, msq, m2, ALU.subtract)
    nc.scalar.activation(var, var, AF.Sqrt, bias=eps1, scale=1.0)
    nc.vector.reciprocal(var, var)
    out = rows_pool.tile([1, n], f32, name=f"{name}_out", tag="rows", bufs=6)
    nc.vector.tensor_scalar(
        out, row, scalar1=mean, scalar2=var, op0=ALU.subtract, op1=ALU.mult
    )
    nc.vector.tensor_tensor(out, out, g_row, ALU.mult)
    nc.vector.tensor_tensor(out, out, b_row, ALU.add)
    return out


def build_kernel():
    import concourse.tile_utils as tile_utils

    tile_utils.max_sbuf_usage = 206 * 1024

    nc = bacc.Bacc()

    # ---- DRAM I/O ----
    xT = nc.dram_tensor("xT", [H, T], f32, kind="ExternalInput")
    ent = nc.dram_tensor("ent", [NE, H], f32, kind="ExternalInput")
    entT = nc.dram_tensor("entT", [H, NE], f32, kind="ExternalInput")
    ee_w1 = nc.dram_tensor("ee_w1", [H, H2], f32, kind="ExternalInput")
    ee_b1 = nc.dram_tensor("ee_b1", [1, H2], f32, kind="ExternalInput")
    ee_w2 = nc.dram_tensor("ee_w2", [H2, H], f32, kind="ExternalInput")
    ee_b2 = nc.dram_tensor("ee_b2", [1, H], f32, kind="ExternalInput")
    ee_g = nc.dram_tensor("ee_g", [1, H], f32, kind="ExternalInput")
    ee_beta = nc.dram_tensor("ee_beta", [1, H], f32, kind="ExternalInput")
    re_w1 = nc.dram_tensor("re_w1", [H, H], f32, kind="ExternalInput")
    re_b1 = nc.dram_tensor("re_b1", [1, H], f32, kind="ExternalInput")
    re_w2 = nc.dram_tensor("re_w2", [H, H], f32, kind="ExternalInput")
    re_b2 = nc.dram_tensor("re_b2", [1, H], f32, kind="ExternalInput")
    re_g = nc.dram_tensor("re_g", [1, H], f32, kind="ExternalInput")
    re_beta = nc.dram_tensor("re_beta", [1, H], f32, kind="ExternalInput")
    rn_w1 = nc.dram_tensor("rn_w1", [3 * H, H2], f32, kind="ExternalInput")
    rn_b1 = nc.dram_tensor("rn_b1", [1, H2], f32, kind="ExternalInput")
    rn_w2 = nc.dram_tensor("rn_w2", [H2, H], f32, kind="ExternalInput")
    rn_b2 = nc.dram_tensor("rn_b2", [1, H], f32, kind="ExternalInput")
    rn_g = nc.dram_tensor("rn_g", [1, H], f32, kind="ExternalInput")
    rn_beta = nc.dram_tensor("rn_beta", [1, H], f32, kind="ExternalInput")
    vn_w1 = nc.dram_tensor("vn_w1", [H, H // 2], f32, kind="ExternalInput")
    vn_b1 = nc.dram_tensor("vn_b1", [1, H // 2], f32, kind="ExternalInput")
    vn_w2 = nc.dram_tensor("vn_w2", [H // 2, 1], f32, kind="ExternalInput")
    vn_b2 = nc.dram_tensor("vn_b2", [1, 1], f32, kind="ExternalInput")

    ef_out = nc.dram_tensor("ef_out", [T, H], f32, kind="ExternalOutput")
    rf_out = nc.dram_tensor("rf_out", [1, H], f32, kind="ExternalOutput")
    ret_out = nc.dram_tensor("ret_out", [TOPK, H], f32, kind="ExternalOutput")
    sims_out = nc.dram_tensor("sims_out", [1, NE], f32, kind="ExternalOutput")
    idx_out = nc.dram_tensor("idx_out", [1, TOPK], i32, kind="ExternalOutput")
    ro_out = nc.dram_tensor("ro_out", [1, H], f32, kind="ExternalOutput")
    vs_out = nc.dram_tensor("vs_out", [1, 1], f32, kind="ExternalOutput")

    from contextlib import ExitStack

    with tile.TileContext(nc) as tc, ExitStack() as stack:
        # ---------- persistent pools ----------
        cpool = stack.enter_context(tc.tile_pool(name="consts", bufs=1))
        rows = stack.enter_context(tc.tile_pool(name="rows", bufs=3))
        scal_p = stack.enter_context(tc.tile_pool(name="scal", bufs=12))
        colp = stack.enter_context(tc.tile_pool(name="colp", bufs=1))

        ident = cpool.tile([P, P], f32)
        make_identity(nc, ident)
        b1_col = cpool.tile([P, MO1], f32)
        nc.sync.dma_start(out=b1_col, in_=ee_b1.rearrange("a (m p) -> p (a m)", p=P))
        b2_bc = cpool.tile([P, H], f32)
        nc.sync.dma_start(out=b2_bc, in_=ee_b2[:, :].to_broadcast([P, H]))
        g_bc = cpool.tile([P, H], f32)
        nc.sync.dma_start(out=g_bc, in_=ee_g[:, :].to_broadcast([P, H]))
        beta_bc = cpool.tile([P, H], f32)
        nc.sync.dma_start(out=beta_bc, in_=ee_beta[:, :].to_broadcast([P, H]))
        eps_t = cpool.tile([P, 1], f32)
        nc.vector.memset(eps_t, EPS)
        eps1 = cpool.tile([1, 1], f32)
        nc.vector.memset(eps1, EPS)
        ones_row = cpool.tile([1, P], f32)
        nc.vector.memset(ones_row, 1.0)
        one1 = cpool.tile([1, 1], f32)
        nc.vector.memset(one1, 1.0)
        pooled_acc = cpool.tile([P, KO1], f32)
        nc.vector.memset(pooled_acc, 0.0)

        def row_tile(n, nm):
            if n > 1024:
                return rows.tile([1, n], f32, name=nm, tag="rows2", bufs=2)
            return rows.tile([1, n], f32, name=nm, tag="rows", bufs=6)

        def load_row(dram, n, nm):
            t_ = row_tile(n, nm)
            nc.sync.dma_start(out=t_, in_=dram[:, :])
            return t_

        # ---------- fused phase A + interleaved vector chain ----------
        with (
            tc.tile_pool(name="w1p", bufs=1) as w1p,
            tc.tile_pool(name="w2rp", bufs=2) as w2rp,
            tc.tile_pool(name="xtp", bufs=1) as xtp,
            tc.tile_pool(name="htp", bufs=1) as htp,
            tc.tile_pool(name="efp", bufs=4) as efp,
            tc.tile_pool(name="lnp", bufs=4) as lnp,
            tc.tile_pool(name="ppp", bufs=2) as ppp,
            tc.tile_pool(name="rewp", bufs=2) as rewp,
            tc.tile_pool(name="etsp", bufs=2) as etsp,
            tc.tile_pool(name="psmm", bufs=2, space="PSUM") as psmm,
            tc.tile_pool(name="psacc", bufs=4, space="PSUM") as psacc,
            tc.tile_pool(name="psva", bufs=1, space="PSUM") as psva,
            tc.tile_pool(name="psca", bufs=1, space="PSUM") as psca,
        ):
            NTT = TB // P

            # --- early pooled pass: mean over tokens, exact f32, 2-stage ---
            for c in range(16):
                pch = ppp.tile([P, KO1, P], f32, name="pch", tag="pch")
                nc.sync.dma_start(
                    out=pch,
                    in_=xT[:, c * P : (c + 1) * P].rearrange(
                        "(ko p) t -> p ko t", p=P
                    ),
                )
                red = lnp.tile([P, KO1], f32, name="red", tag="red")
                nc.vector.tensor_reduce(
                    red, pch, axis=mybir.AxisListType.X, op=ALU.add
                )
                nc.vector.tensor_tensor(pooled_acc, pooled_acc, red, ALU.add)
            pooled_col = colp.tile([P, KO1], f32, name="pooled_col", tag="pooled_col")
            nc.scalar.mul(pooled_col, pooled_acc, 1.0 / T)

            # --- W1 resident as f32r ---
            w1r = []
            for ko in range(KO1):
                wr = w1p.tile([P, H2], f32r, name=f"w1r{ko}", tag=f"w1r{ko}")
                nc.sync.dma_start(
                    out=wr, in_=ee_w1[ko * P : (ko + 1) * P, :].bitcast(f32r)
                )
                w1r.append(wr)

            # --- chain segment builders (emitted between blocks) ---
            def vec_mlp_layer(w_dram, in_col, n_out, bias_row, nm):
                """[1, n_out] = in_col.T @ W + bias, fp32, weights streamed."""
                out_row = row_tile(n_out, f"{nm}_row")
                nko = in_col.shape[1]
                for n in range((n_out + 511) // 512):
                    nsz = min(512, n_out - n * 512)
                    ps_a = psva.tile([1, 512], f32, name=f"{nm}ps{n}", tag="vecacc")
                    for ko in range(nko):
                        wch = rewp.tile([P, 512], f32, name=f"{nm}w", tag="rew")
                        nc.sync.dma_start(
                            out=wch[:, :nsz],
                            in_=w_dram[
                                ko * P : (ko + 1) * P, n * 512 : n * 512 + nsz
                            ],
                        )
                        nc.tensor.matmul(
                            ps_a[:, :nsz],
                            lhsT=in_col[:, ko : ko + 1],
                            rhs=wch[:, :nsz],
                            start=(ko == 0),
                            stop=(ko == nko - 1),
                        )
                    nc.vector.tensor_tensor(
                        out_row[:, n * 512 : n * 512 + nsz],
                        ps_a[:, :nsz],
                        bias_row[:, n * 512 : n * 512 + nsz],
                        ALU.add,
                    )
                return out_row

            def row_to_col(row, n_chunks, nm):
                ps_col = psca.tile([P, 16], f32, name=f"{nm}_ps", tag="colacc")
                for ko in range(n_chunks):
                    nc.tensor.matmul(
                        ps_col[:, ko : ko + 1],
                        lhsT=row[:, ko * P : (ko + 1) * P],
                        rhs=one1,
                        start=True,
                        stop=True,
                    )
                col = colp.tile([P, n_chunks], f32, name=f"{nm}_col", tag=f"{nm}_col")
                nc.vector.tensor_copy(col, ps_col[:, :n_chunks])
                return col

            state = {}

            def seg_re1():
                reb1 = load_row(re_b1, H, "reb1")
                h1 = vec_mlp_layer(re_w1, pooled_col, H, reb1, "re1")
                nc.scalar.activation(h1, h1, AF.Gelu)
                state["g1_col"] = row_to_col(h1, KO1, "g1")

            def seg_re2():
                reb2 = load_row(re_b2, H, "reb2")
                r_row = vec_mlp_layer(re_w2, state["g1_col"], H, reb2, "re2")
                reg_row = load_row(re_g, H, "reg")
                rebeta_row = load_row(re_beta, H, "rebeta")
                rf_row = _row_layernorm(
                    nc, rows, scal_p, r_row, reg_row, rebeta_row, eps1, "rf"
                )
                nc.sync.dma_start(out=rf_out[:, :], in_=rf_row)
                state["rf_col"] = row_to_col(rf_row, KO1, "rf")

            def seg_sims():
                rf_col = state["rf_col"]
                sims_row = row_tile(NE, "sims_row")
                for n in range(2):
                    nsz = 512 if n == 0 else NE - 512
                    ps_s = psva.tile([1, 512], f32, name=f"simsps{n}", tag="vecacc")
                    for ko in range(KO1):
                        ech = etsp.tile([P, 512], f32, name="ech", tag="ets")
                        nc.sync.dma_start(
                            out=ech[:, :nsz],
                            in_=entT[
                                ko * P : (ko + 1) * P, n * 512 : n * 512 + nsz
                            ],
                        )
                        nc.tensor.matmul(
                            ps_s[:, :nsz],
                            lhsT=rf_col[:, ko : ko + 1],
                            rhs=ech[:, :nsz],
                            start=(ko == 0),
                            stop=(ko == KO1 - 1),
                        )
                    nc.vector.tensor_copy(
                        sims_row[:, n * 512 : n * 512 + nsz], ps_s[:, :nsz]
                    )
                nc.sync.dma_start(out=sims_out[:, :], in_=sims_row)
                mxv = colp.tile([1, 8], f32, name="mxv", tag="mxv")
                mxi = colp.tile([1, 8], u32, name="mxi", tag="mxi")
                nc.vector.max_with_indices(mxv, mxi, sims_row)
                mii = colp.tile([1, 8], i32, name="mii", tag="mii")
                nc.vector.tensor_copy(mii, mxi)
                nc.sync.dma_start(out=idx_out[:, :], in_=mii[:, :TOPK])
                mif = colp.tile([1, 8], f32, name="mif", tag="mif")
                nc.vector.tensor_copy(mif, mxi)
                ps_ib = psca.tile([P, 8], f32, name="ps_ib", tag="colacc")
                nc.tensor.matmul(
                    ps_ib, lhsT=ones_row, rhs=mif, start=True, stop=True
                )
                idx_b = colp.tile([P, 8], f32, name="idx_b", tag="idx_b")
                nc.vector.tensor_copy(idx_b, ps_ib)
                iota_i = colp.tile([P, 8], i32, name="iota_i", tag="iota_i")
                nc.gpsimd.iota(
                    iota_i, pattern=[[P, 8]], base=0, channel_multiplier=1
                )
                iota_f = colp.tile([P, 8], f32, name="iota_f", tag="iota_f")
                nc.vector.tensor_copy(iota_f, iota_i)
                onehot = colp.tile([P, 8, TOPK], f32, name="onehot", tag="onehot")
                for et in range(8):
                    nc.vector.tensor_tensor(
                        onehot[:, et, :],
                        idx_b[:, :TOPK],
                        iota_f[:, et : et + 1].to_broadcast([P, TOPK]),
                        ALU.is_equal,
                    )
                state["onehot"] = onehot

            segments = [seg_re1, seg_re2, seg_sims]

            # --- phase A blocks with interleaved segments ---
            for b in range(NBLK):
                xt = xtp.tile([P, KO1, TB], f32r, name="xt", tag="xt")
                nc.sync.dma_start(
                    out=xt,
                    in_=xT[:, b * TB : (b + 1) * TB]
                    .rearrange("(ko p) t -> p ko t", p=P)
                    .bitcast(f32r),
                )
                ht = htp.tile([P, MO1, TB], f32r, name="ht", tag="ht")
                for m in range(MO1):
                    ps1 = psmm.tile([P, TB], f32, name="ps1", tag="mm")
                    for ko in range(KO1):
                        nc.tensor.matmul(
                            ps1,
                            lhsT=w1r[ko][:, m * P : (m + 1) * P],
                            rhs=xt[:, ko, :],
                            start=(ko == 0),
                            stop=(ko == KO1 - 1),
                        )
                    nc.scalar.activation(
                        ht[:, m, :], ps1, AF.Gelu, bias=b1_col[:, m : m + 1], scale=1.0
                    )

                ef_blk = [
                    efp.tile([P, H], f32, name=f"ef{tt}", tag="ef")
                    for tt in range(NTT)
                ]
                for g in range(2):
                    accs = [
                        psacc.tile([P, 512], f32, name=f"acc{tt}", tag="acc")
                        for tt in range(NTT)
                    ]
                    for ko in range(KO2):
                        w2g = w2rp.tile([P, 512], f32r, name="w2g", tag="w2g")
                        nc.sync.dma_start(
                            out=w2g,
                            in_=ee_w2[
                                ko * P : (ko + 1) * P, g * 512 : (g + 1) * 512
                            ].bitcast(f32r),
                        )
                        for tt in range(NTT):
                            nc.tensor.matmul(
                                accs[tt],
                                lhsT=ht[:, ko, tt * P : (tt + 1) * P],
                                rhs=w2g,
                                start=(ko == 0),
                                stop=(ko == KO2 - 1),
                            )
                    for tt in range(NTT):
                        nc.vector.tensor_tensor(
                            ef_blk[tt][:, g * 512 : (g + 1) * 512],
                            accs[tt],
                            b2_bc[:, g * 512 : (g + 1) * 512],
                            ALU.add,
                        )

                for tt in range(NTT):
                    ef_t = ef_blk[tt]
                    stats = lnp.tile([P, 2, 6], f32, name="stats", tag="stats")
                    for sg in range(2):
                        nc.vector.bn_stats(
                            stats[:, sg, :], ef_t[:, sg * 512 : (sg + 1) * 512]
                        )
                    mv = lnp.tile([P, 2], f32, name="mv", tag="mv")
                    nc.vector.bn_aggr(mv, stats)
                    nc.scalar.activation(
                        mv[:, 1:2], mv[:, 1:2], AF.Sqrt, bias=eps_t, scale=1.0
                    )
                    nc.vector.reciprocal(mv[:, 1:2], mv[:, 1:2])
                    nc.vector.tensor_scalar(
                        ef_t,
                        ef_t,
                        scalar1=mv[:, 0:1],
                        scalar2=mv[:, 1:2],
                        op0=ALU.subtract,
                        op1=ALU.mult,
                    )
                    nc.vector.tensor_tensor(ef_t, ef_t, g_bc, ALU.mult)
                    nc.vector.tensor_tensor(ef_t, ef_t, beta_bc, ALU.add)
                    nc.sync.dma_start(
                        out=ef_out[b * TB + tt * P : b * TB + (tt + 1) * P, :],
                        in_=ef_t,
                    )

                if b < len(segments):
                    segments[b]()

        # ---------- post: reasoning + validation ----------
        with (
            tc.tile_pool(name="entp", bufs=1) as entp,
            tc.tile_pool(name="rnwp", bufs=6) as rnwp,
            tc.tile_pool(name="vnwp", bufs=2) as vnwp,
            tc.tile_pool(name="psb_va", bufs=6, space="PSUM") as psb_va,
            tc.tile_pool(name="psb_ca", bufs=2, space="PSUM") as psb_ca,
        ):
            # --- gather retrieved entities (resident table, exact fp32) ---
            onehot = state["onehot"]
            ent_nat = []
            for et in range(8):
                rows_e = P if et < 7 else NE - 7 * P
                t_ = entp.tile([P, H], f32, name=f"ent{et}", tag=f"ent{et}")
                nc.sync.dma_start(
                    out=t_[:rows_e, :], in_=ent[et * P : et * P + rows_e, :]
                )
                ent_nat.append((t_, rows_e))
            ret_col = colp.tile([P, KO1, TOPK], f32, name="ret_col", tag="ret_col")
            for fo in range(KO1):
                ps_g = psb_ca.tile([P, TOPK], f32, name="ps_g", tag="colacc")
                for et in range(8):
                    t_, rows_e = ent_nat[et]
                    nc.tensor.matmul(
                        ps_g,
                        lhsT=t_[:rows_e, fo * P : (fo + 1) * P],
                        rhs=onehot[:rows_e, et, :],
                        start=(et == 0),
                        stop=(et == 7),
                    )
                nc.vector.tensor_copy(ret_col[:, fo, :], ps_g)
            ret_row = rows.tile([TOPK, H], f32, name="ret_row", tag="rows", bufs=6)
            for fo in range(KO1):
                ps_r5 = psb_ca.tile([TOPK, P], f32, name="ps_r5", tag="colacc")
                nc.tensor.transpose(ps_r5, ret_col[:, fo, :], ident)
                nc.vector.tensor_copy(ret_row[:, fo * P : (fo + 1) * P], ps_r5)
            nc.sync.dma_start(out=ret_out[:, :], in_=ret_row)
            ev_col = colp.tile([P, KO1], f32, name="ev_col", tag="ev_col")
            nc.vector.tensor_reduce(
                ev_col, ret_col, axis=mybir.AxisListType.X, op=ALU.add
            )
            nc.scalar.mul(ev_col, ev_col, 1.0 / TOPK)
            rin_col = colp.tile([P, 24], f32r, name="rin_col", tag="rin_col")
            nc.vector.tensor_copy(rin_col[:, 0:8], ev_col)
            nc.vector.tensor_copy(rin_col[:, 8:16], state["rf_col"])
            nc.vector.tensor_copy(rin_col[:, 16:24], pooled_col)

            def row_to_col2(row, n_chunks, nm):
                ps_col = psb_ca.tile([P, 16], f32, name=f"{nm}_ps", tag="colacc")
                for ko in range(n_chunks):
                    nc.tensor.matmul(
                        ps_col[:, ko : ko + 1],
                        lhsT=row[:, ko * P : (ko + 1) * P],
                        rhs=one1,
                        start=True,
                        stop=True,
                    )
                col = colp.tile([P, n_chunks], f32, name=f"{nm}_col", tag=f"{nm}_col")
                nc.vector.tensor_copy(col, ps_col[:, :n_chunks])
                return col

            rnb1 = load_row(rn_b1, H2, "rnb1")
            h1r_row = row_tile(H2, "h1r_row")
            ps_rn = [
                psb_va.tile([1, 512], f32, name=f"ps_rn{n}", tag="vecacc")
                for n in range(4)
            ]
            for ko in range(24):
                rwr = rnwp.tile([P, H2], f32r, name="rnw", tag="rnw")
                nc.sync.dma_start(
                    out=rwr, in_=rn_w1[ko * P : (ko + 1) * P, :].bitcast(f32r)
                )
                for n in range(4):
                    nc.tensor.matmul(
                        ps_rn[n],
                        lhsT=rin_col[:, ko : ko + 1],
                        rhs=rwr[:, n * 512 : (n + 1) * 512],
                        start=(ko == 0),
                        stop=(ko == 23),
                    )
            for n in range(4):
                nc.vector.tensor_tensor(
                    h1r_row[:, n * 512 : (n + 1) * 512],
                    ps_rn[n],
                    rnb1[:, n * 512 : (n + 1) * 512],
                    ALU.add,
                )
            nc.scalar.activation(h1r_row, h1r_row, AF.Gelu)
            g1r_col = row_to_col2(h1r_row, 16, "g1r")
            g1r_r = colp.tile([P, 16], f32r, name="g1r_r", tag="g1r_r")
            nc.vector.tensor_copy(g1r_r, g1r_col)

            rnb2 = load_row(rn_b2, H, "rnb2")
            r2_row = row_tile(H, "r2_row")
            ps_rn2 = [
                psb_va.tile([1, 512], f32, name=f"ps_rn2{n}", tag="vecacc")
                for n in range(2)
            ]
            for ko in range(16):
                rwr = rnwp.tile([P, H], f32r, name="rnw2", tag="rnw")
                nc.sync.dma_start(
                    out=rwr, in_=rn_w2[ko * P : (ko + 1) * P, :].bitcast(f32r)
                )
                for n in range(2):
                    nc.tensor.matmul(
                        ps_rn2[n],
                        lhsT=g1r_r[:, ko : ko + 1],
                        rhs=rwr[:, n * 512 : (n + 1) * 512],
                        start=(ko == 0),
                        stop=(ko == 15),
                    )
            for n in range(2):
                nc.vector.tensor_tensor(
                    r2_row[:, n * 512 : (n + 1) * 512],
                    ps_rn2[n],
                    rnb2[:, n * 512 : (n + 1) * 512],
                    ALU.add,
                )
            rng_row = load_row(rn_g, H, "rng_row")
            rnbeta_row = load_row(rn_beta, H, "rnbeta_row")
            ro_row = _row_layernorm(
                nc, rows, scal_p, r2_row, rng_row, rnbeta_row, eps1, "ro"
            )
            nc.sync.dma_start(out=ro_out[:, :], in_=ro_row)

            ro_col = row_to_col2(ro_row, KO1, "roc")
            vnb1_row = load_row(vn_b1, H // 2, "vnb1_row")
            ps_v = psb_va.tile([1, 512], f32, name="ps_v", tag="vecacc")
            for ko in range(KO1):
                vw = vnwp.tile([P, 512], f32, name="vnw", tag="vnw")
                nc.sync.dma_start(out=vw, in_=vn_w1[ko * P : (ko + 1) * P, :])
                nc.tensor.matmul(
                    ps_v,
                    lhsT=ro_col[:, ko : ko + 1],
                    rhs=vw,
                    start=(ko == 0),
                    stop=(ko == KO1 - 1),
                )
            v1_row = row_tile(512, "v1_row")
            nc.vector.tensor_tensor(v1_row, ps_v, vnb1_row, ALU.add)
            nc.scalar.activation(v1_row, v1_row, AF.Gelu)
            v1_col = row_to_col2(v1_row, 4, "v1")
            vnw2 = colp.tile([P, 4, 1], f32, name="vnw2", tag="vnw2")
            nc.sync.dma_start(
                out=vnw2, in_=vn_w2.rearrange("(ko p) n -> p ko n", p=P)
            )
            vnb2_row = scal_p.tile([1, 1], f32, name="vnb2_row", tag="scal")
            nc.sync.dma_start(out=vnb2_row, in_=vn_b2[:, :])
            ps_v2 = psb_va.tile([1, 1], f32, name="ps_v2", tag="vecacc")
            for ko in range(4):
                nc.tensor.matmul(
                    ps_v2,
                    lhsT=v1_col[:, ko : ko + 1],
                    rhs=vnw2[:, ko, :],
                    start=(ko == 0),
                    stop=(ko == 3),
                )
            v2_row = scal_p.tile([1, 1], f32, name="v2_row", tag="scal")
            nc.vector.tensor_tensor(v2_row, ps_v2, vnb2_row, ALU.add)
            nc.scalar.activation(v2_row, v2_row, AF.Sigmoid)
            nc.sync.dma_start(out=vs_out[:, :], in_=v2_row)

    nc.compile()
    return nc


_NC_CACHE = None


def _get_nc():
    global _NC_CACHE
    if _NC_CACHE is None:
        _NC_CACHE = build_kernel()
    return _NC_CACHE


def run(inputs, trace=False):
    from concourse.bass_utils import run_bass_kernel_spmd

    nc = _get_nc()
    hs = np.ascontiguousarray(inputs["hidden_states"], dtype=np.float32)
    B = hs.shape[0]
    assert B == N_CORES

    def row(name, n):
        return np.ascontiguousarray(
            np.asarray(inputs[name], dtype=np.float32).reshape(1, n)
        )

    E = np.ascontiguousarray(inputs["entity_embeddings"], np.float32)
    shared = {
        "ent": E,
        "entT": np.ascontiguousarray(E.T),
        "ee_w1": np.ascontiguousarray(inputs["ee_w1"], np.float32),
        "ee_b1": row("ee_b1", H2),
        "ee_w2": np.ascontiguousarray(inputs["ee_w2"], np.float32),
        "ee_b2": row("ee_b2", H),
        "ee_g": row("ee_g", H),
        "ee_beta": row("ee_beta", H),
        "re_w1": np.ascontiguousarray(inputs["re_w1"], np.float32),
        "re_b1": row("re_b1", H),
        "re_w2": np.ascontiguousarray(inputs["re_w2"], np.float32),
        "re_b2": row("re_b2", H),
        "re_g": row("re_g", H),
        "re_beta": row("re_beta", H),
        "rn_w1": np.ascontiguousarray(inputs["rn_w1"], np.float32),
        "rn_b1": row("rn_b1", H2),
        "rn_w2": np.ascontiguousarray(inputs["rn_w2"], np.float32),
        "rn_b2": row("rn_b2", H),
        "rn_g": row("rn_g", H),
        "rn_beta": row("rn_beta", H),
        "vn_w1": np.ascontiguousarray(inputs["vn_w1"], np.float32),
        "vn_b1": row("vn_b1", H // 2),
        "vn_w2": np.ascontiguousarray(inputs["vn_w2"], np.float32),
        "vn_b2": row("vn_b2", 1),
    }
    in_maps = [
        dict(shared, xT=np.ascontiguousarray(hs[c].T)) for c in range(B)
    ]
    res = run_bass_kernel_spmd(
        nc, in_maps, core_ids=list(range(N_CORES)), trace=trace
    )
    r = res.results
    ef = np.stack([r[c]["ef_out"] for c in range(B)])
    rf = np.stack([r[c]["rf_out"][0] for c in range(B)])
    ret = np.stack([r[c]["ret_out"] for c in range(B)])
    sims = np.stack([r[c]["sims_out"][0] for c in range(B)])
    idx = np.stack([r[c]["idx_out"][0] for c in range(B)]).astype(np.int32)
    ro = np.stack([r[c]["ro_out"][0] for c in range(B)])
    vs = np.stack([r[c]["vs_out"][0] for c in range(B)])
    out = (ef, rf, ret, sims, idx, ro, vs)
    return (out, res) if trace else out


def kernel(**inputs):
    return run(inputs, trace=False)


# revision 29
# speedup vs baseline: 1.2719x; 1.2719x over previous
"""Trainium2 Bass kernel for nn_KnowledgeBaseModule.

Data-parallel over batch: 8 batch rows -> 8 NeuronCores, weights/tables
replicated. Entity encoder (the dominant compute) runs in feature-major
(transposed) layout so both weight matmuls use natural weight layouts; the
two big matmuls use float32r (fast fp32) operands, everything index-critical
stays fp32.
"""

import numpy as np

import concourse.bass as bass
import concourse.mybir as mybir
import concourse.tile as tile
from concourse import bacc
from concourse.masks import make_identity

f32 = mybir.dt.float32
f32r = mybir.dt.float32r
i32 = mybir.dt.int32
u32 = mybir.dt.uint32

P = 128
H = 1024
H2 = 2048
T = 2048          # tokens per core (one batch row)
TB = 512          # token block
NBLK = T // TB    # 4
NE = 1000
TOPK = 5
EPS = 1e-5
KO1 = H // P      # 8
MO1 = H2 // P     # 16
KO2 = H2 // P     # 16
MO2 = H // P      # 8
N_CORES = 8

AF = mybir.ActivationFunctionType
ALU = mybir.AluOpType


def _row_to_col(nc, ps_pool, one1, col_pool, row, n_chunks, name):
    """[1, n_chunks*128] row -> [128, n_chunks] column layout via K=1 matmuls."""
    ps_col = ps_pool.tile([P, n_chunks], f32, name=f"{name}_ps", tag="colacc")
    for ko in range(n_chunks):
        nc.tensor.matmul(
            ps_col[:, ko : ko + 1],
            lhsT=row[:, ko * P : (ko + 1) * P],
            rhs=one1,
            start=True,
            stop=True,
        )
    col = col_pool.tile([P, n_chunks], f32, name=f"{name}_col", tag=f"{name}_col")
    nc.vector.tensor_copy(col, ps_col)
    return col


def _row_layernorm(nc, rows_pool, scal_pool, row, g_row, b_row, eps1, name):
    """LayerNorm along the free dim of a [1, N] row (single partition)."""
    n = row.shape[1]

    def scal(nm):
        return scal_pool.tile([1, 1], f32, name=f"{name}_{nm}", tag="scal")

    s1 = scal("s1")
    s2 = scal("s2")
    sq = rows_pool.tile([1, n], f32, name=f"{name}_sq", tag="rows", bufs=6)
    nc.vector.tensor_reduce(s1, row, axis=mybir.AxisListType.X, op=ALU.add)
    nc.vector.tensor_tensor(sq, row, row, ALU.mult)
    nc.vector.tensor_reduce(s2, sq, axis=mybir.AxisListType.X, op=ALU.add)
    mean = scal("mean")
    msq = scal("msq")
    nc.scalar.mul(mean, s1, 1.0 / n)
    nc.scalar.mul(msq, s2, 1.0 / n)
    m2 = scal("m2")
    var = scal("var")
    nc.vector.tensor_tensor(m2, mean, mean, ALU.mult)
    nc.vector.tensor_tensor(var, msq, m2, ALU.subtract)
    nc.scalar.activation(var, var, AF.Sqrt, bias=eps1, scale=1.0)
    nc.vector.reciprocal(var, var)
    out = rows_pool.tile([1, n], f32, name=f"{name}_out", tag="rows", bufs=6)
    nc.vector.tensor_scalar(
        out, row, scalar1=mean, scalar2=var, op0=ALU.subtract, op1=ALU.mult
    )
    nc.vector.tensor_tensor(out, out, g_row, ALU.mult)
    nc.vector.tensor_tensor(out, out, b_row, ALU.add)
    return out


def build_kernel():
    import concourse.tile_utils as tile_utils

    tile_utils.max_sbuf_usage = 208 * 1024

    nc = bacc.Bacc()

    # ---- DRAM I/O ----
    xT = nc.dram_tensor("xT", [H, T], f32, kind="ExternalInput")
    ent = nc.dram_tensor("ent", [NE, H], f32, kind="ExternalInput")
    entT = nc.dram_tensor("entT", [H, NE], f32, kind="ExternalInput")
    ee_w1 = nc.dram_tensor("ee_w1", [H, H2], f32, kind="ExternalInput")
    ee_b1 = nc.dram_tensor("ee_b1", [1, H2], f32, kind="ExternalInput")
    ee_w2 = nc.dram_tensor("ee_w2", [H2, H], f32, kind="ExternalInput")
    ee_b2 = nc.dram_tensor("ee_b2", [1, H], f32, kind="ExternalInput")
    ee_g = nc.dram_tensor("ee_g", [1, H], f32, kind="ExternalInput")
    ee_beta = nc.dram_tensor("ee_beta", [1, H], f32, kind="ExternalInput")
    re_w1 = nc.dram_tensor("re_w1", [H, H], f32, kind="ExternalInput")
    re_b1 = nc.dram_tensor("re_b1", [1, H], f32, kind="ExternalInput")
    re_w2 = nc.dram_tensor("re_w2", [H, H], f32, kind="ExternalInput")
    re_b2 = nc.dram_tensor("re_b2", [1, H], f32, kind="ExternalInput")
    re_g = nc.dram_tensor("re_g", [1, H], f32, kind="ExternalInput")
    re_beta = nc.dram_tensor("re_beta", [1, H], f32, kind="ExternalInput")
    rn_w1 = nc.dram_tensor("rn_w1", [3 * H, H2], f32, kind="ExternalInput")
    rn_b1 = nc.dram_tensor("rn_b1", [1, H2], f32, kind="ExternalInput")
    rn_w2 = nc.dram_tensor("rn_w2", [H2, H], f32, kind="ExternalInput")
    rn_b2 = nc.dram_tensor("rn_b2", [1, H], f32, kind="ExternalInput")
    rn_g = nc.dram_tensor("rn_g", [1, H], f32, kind="ExternalInput")
    rn_beta = nc.dram_tensor("rn_beta", [1, H], f32, kind="ExternalInput")
    vn_w1 = nc.dram_tensor("vn_w1", [H, H // 2], f32, kind="ExternalInput")
    vn_b1 = nc.dram_tensor("vn_b1", [1, H // 2], f32, kind="ExternalInput")
    vn_w2 = nc.dram_tensor("vn_w2", [H // 2, 1], f32, kind="ExternalInput")
    vn_b2 = nc.dram_tensor("vn_b2", [1, 1], f32, kind="ExternalInput")

    ef_out = nc.dram_tensor("ef_out", [T, H], f32, kind="ExternalOutput")
    rf_out = nc.dram_tensor("rf_out", [1, H], f32, kind="ExternalOutput")
    ret_out = nc.dram_tensor("ret_out", [TOPK, H], f32, kind="ExternalOutput")
    sims_out = nc.dram_tensor("sims_out", [1, NE], f32, kind="ExternalOutput")
    idx_out = nc.dram_tensor("idx_out", [1, TOPK], i32, kind="ExternalOutput")
    ro_out = nc.dram_tensor("ro_out", [1, H], f32, kind="ExternalOutput")
    vs_out = nc.dram_tensor("vs_out", [1, 1], f32, kind="ExternalOutput")

    from contextlib import ExitStack

    with tile.TileContext(nc) as tc, ExitStack() as stack:
        # ---------- persistent pools ----------
        cpool = stack.enter_context(tc.tile_pool(name="consts", bufs=1))
        colp = stack.enter_context(tc.tile_pool(name="colp", bufs=1))
        rows = None
        scal_p = None

        ident = cpool.tile([P, P], f32)
        make_identity(nc, ident)
        b1_col = cpool.tile([P, MO1], f32)
        nc.sync.dma_start(out=b1_col, in_=ee_b1.rearrange("a (m p) -> p (a m)", p=P))
        b2_bc = cpool.tile([P, H], f32)
        nc.sync.dma_start(out=b2_bc, in_=ee_b2[:, :].to_broadcast([P, H]))
        g_bc = cpool.tile([P, H], f32)
        nc.sync.dma_start(out=g_bc, in_=ee_g[:, :].to_broadcast([P, H]))
        beta_bc = cpool.tile([P, H], f32)
        nc.sync.dma_start(out=beta_bc, in_=ee_beta[:, :].to_broadcast([P, H]))
        eps_t = cpool.tile([P, 1], f32)
        nc.vector.memset(eps_t, EPS)
        eps1 = cpool.tile([1, 1], f32)
        nc.vector.memset(eps1, EPS)
        ones_row = cpool.tile([1, P], f32)
        nc.vector.memset(ones_row, 1.0)
        one1 = cpool.tile([1, 1], f32)
        nc.vector.memset(one1, 1.0)
        pooled_acc = cpool.tile([P, KO1], f32)
        nc.vector.memset(pooled_acc, 0.0)

        def row_tile(n, nm):
            if n > 1024:
                return rows.tile([1, n], f32, name=nm, tag="rows2", bufs=2)
            return rows.tile([1, n], f32, name=nm, tag="rows", bufs=6)

        def load_row(dram, n, nm):
            t_ = row_tile(n, nm)
            nc.sync.dma_start(out=t_, in_=dram[:, :])
            return t_

        # ---------- fused phase A + interleaved vector chain ----------
        with (
            tc.tile_pool(name="w1p", bufs=1) as w1p,
            tc.tile_pool(name="w2p", bufs=1) as w2p,
            tc.tile_pool(name="xtp", bufs=1) as xtp,
            tc.tile_pool(name="htp", bufs=1) as htp,
            tc.tile_pool(name="efp", bufs=4) as efp,
            tc.tile_pool(name="lnp", bufs=4) as lnp,
            tc.tile_pool(name="psmm", bufs=6, space="PSUM") as psmm,
        ):
            NTT = TB // P

            # --- W1 + W2 resident as f32r ---
            w1r = []
            for ko in range(KO1):
                wr = w1p.tile([P, H2], f32r, name=f"w1r{ko}", tag=f"w1r{ko}")
                nc.sync.dma_start(
                    out=wr, in_=ee_w1[ko * P : (ko + 1) * P, :].bitcast(f32r)
                )
                w1r.append(wr)
            w2r = []
            for ko in range(KO2):
                wr = w2p.tile([P, H], f32r, name=f"w2r{ko}", tag=f"w2r{ko}")
                nc.sync.dma_start(
                    out=wr, in_=ee_w2[ko * P : (ko + 1) * P, :].bitcast(f32r)
                )
                w2r.append(wr)

            # --- chain segment builders (emitted between blocks) ---
            def vec_mlp_layer(w_dram, in_col, n_out, bias_row, nm):
                """[1, n_out] = in_col.T @ W + bias, fp32, weights streamed."""
                out_row = row_tile(n_out, f"{nm}_row")
                nko = in_col.shape[1]
                for n in range((n_out + 511) // 512):
                    nsz = min(512, n_out - n * 512)
                    ps_a = psva.tile([1, 512], f32, name=f"{nm}ps{n}", tag="vecacc")
                    for ko in range(nko):
                        wch = rewp.tile([P, 512], f32, name=f"{nm}w", tag="rew")
                        nc.sync.dma_start(
                            out=wch[:, :nsz],
                            in_=w_dram[
                                ko * P : (ko + 1) * P, n * 512 : n * 512 + nsz
                            ],
                        )
                        nc.tensor.matmul(
                            ps_a[:, :nsz],
                            lhsT=in_col[:, ko : ko + 1],
                            rhs=wch[:, :nsz],
                            start=(ko == 0),
                            stop=(ko == nko - 1),
                        )
                    nc.vector.tensor_tensor(
                        out_row[:, n * 512 : n * 512 + nsz],
                        ps_a[:, :nsz],
                        bias_row[:, n * 512 : n * 512 + nsz],
                        ALU.add,
                    )
                return out_row

            def row_to_col(row, n_chunks, nm):
                ps_col = psca.tile([P, 16], f32, name=f"{nm}_ps", tag="colacc")
                for ko in range(n_chunks):
                    nc.tensor.matmul(
                        ps_col[:, ko : ko + 1],
                        lhsT=row[:, ko * P : (ko + 1) * P],
                        rhs=one1,
                        start=True,
                        stop=True,
                    )
                col = colp.tile([P, n_chunks], f32, name=f"{nm}_col", tag=f"{nm}_col")
                nc.vector.tensor_copy(col, ps_col[:, :n_chunks])
                return col

            state = {}

            def seg_re1():
                reb1 = load_row(re_b1, H, "reb1")
                h1 = vec_mlp_layer(re_w1, pooled_col, H, reb1, "re1")
                nc.scalar.activation(h1, h1, AF.Gelu)
                state["g1_col"] = row_to_col(h1, KO1, "g1")

            def seg_re2():
                reb2 = load_row(re_b2, H, "reb2")
                r_row = vec_mlp_layer(re_w2, state["g1_col"], H, reb2, "re2")
                reg_row = load_row(re_g, H, "reg")
                rebeta_row = load_row(re_beta, H, "rebeta")
                rf_row = _row_layernorm(
                    nc, rows, scal_p, r_row, reg_row, rebeta_row, eps1, "rf"
                )
                nc.sync.dma_start(out=rf_out[:, :], in_=rf_row)
                state["rf_col"] = row_to_col(rf_row, KO1, "rf")

            def seg_sims():
                rf_col = state["rf_col"]
                sims_row = row_tile(NE, "sims_row")
                for n in range(2):
                    nsz = 512 if n == 0 else NE - 512
                    ps_s = psva.tile([1, 512], f32, name=f"simsps{n}", tag="vecacc")
                    for ko in range(KO1):
                        ech = etsp.tile([P, 512], f32, name="ech", tag="ets")
                        nc.sync.dma_start(
                            out=ech[:, :nsz],
                            in_=entT[
                                ko * P : (ko + 1) * P, n * 512 : n * 512 + nsz
                            ],
                        )
                        nc.tensor.matmul(
                            ps_s[:, :nsz],
                            lhsT=rf_col[:, ko : ko + 1],
                            rhs=ech[:, :nsz],
                            start=(ko == 0),
                            stop=(ko == KO1 - 1),
                        )
                    nc.vector.tensor_copy(
                        sims_row[:, n * 512 : n * 512 + nsz], ps_s[:, :nsz]
                    )
                nc.sync.dma_start(out=sims_out[:, :], in_=sims_row)
                mxv = colp.tile([1, 8], f32, name="mxv", tag="mxv")
                mxi = colp.tile([1, 8], u32, name="mxi", tag="mxi")
                nc.vector.max_with_indices(mxv, mxi, sims_row)
                mii = colp.tile([1, 8], i32, name="mii", tag="mii")
                nc.vector.tensor_copy(mii, mxi)
                nc.sync.dma_start(out=idx_out[:, :], in_=mii[:, :TOPK])
                mif = colp.tile([1, 8], f32, name="mif", tag="mif")
                nc.vector.tensor_copy(mif, mxi)
                ps_ib = psca.tile([P, 8], f32, name="ps_ib", tag="colacc")
                nc.tensor.matmul(
                    ps_ib, lhsT=ones_row, rhs=mif, start=True, stop=True
                )
                idx_b = colp.tile([P, 8], f32, name="idx_b", tag="idx_b")
                nc.vector.tensor_copy(idx_b, ps_ib)
                iota_i = colp.tile([P, 8], i32, name="iota_i", tag="iota_i")
                nc.gpsimd.iota(
                    iota_i, pattern=[[P, 8]], base=0, channel_multiplier=1
                )
                iota_f = colp.tile([P, 8], f32, name="iota_f", tag="iota_f")
                nc.vector.tensor_copy(iota_f, iota_i)
                onehot = colp.tile([P, 8, TOPK], f32, name="onehot", tag="onehot")
                for et in range(8):
                    nc.vector.tensor_tensor(
                        onehot[:, et, :],
                        idx_b[:, :TOPK],
                        iota_f[:, et : et + 1].to_broadcast([P, TOPK]),
                        ALU.is_equal,
                    )
                state["onehot"] = onehot

            segments = [seg_re1, seg_re2, seg_sims]

            # --- phase A blocks with interleaved segments ---
            pooled_col = colp.tile([P, KO1], f32, name="pooled_col", tag="pooled_col")
            for b in range(NBLK):
                xt = xtp.tile([P, KO1, TB], f32r, name="xt", tag="xt")
                nc.sync.dma_start(
                    out=xt,
                    in_=xT[:, b * TB : (b + 1) * TB]
                    .rearrange("(ko p) t -> p ko t", p=P)
                    .bitcast(f32r),
                )
                # pooled: exact f32 view, 2-stage reduce for short sum chains
                red2 = lnp.tile([P, KO1, 4], f32, name="red2", tag="red2")
                nc.vector.tensor_reduce(
                    red2,
                    xt.bitcast(f32).rearrange("p ko (c t) -> p ko c t", c=4),
                    axis=mybir.AxisListType.X,
                    op=ALU.add,
                )
                red = lnp.tile([P, KO1], f32, name="red", tag="red")
                nc.vector.tensor_reduce(
                    red, red2, axis=mybir.AxisListType.X, op=ALU.add
                )
                nc.vector.tensor_tensor(pooled_acc, pooled_acc, red, ALU.add)
                if b == NBLK - 1:
                    nc.scalar.mul(pooled_col, pooled_acc, 1.0 / T)

                ht = htp.tile([P, MO1, TB], f32r, name="ht", tag="ht")
                for m in range(MO1):
                    ps1 = psmm.tile([P, TB], f32, name="ps1", tag="mm")
                    for ko in range(KO1):
                        nc.tensor.matmul(
                            ps1,
                            lhsT=w1r[ko][:, m * P : (m + 1) * P],
                            rhs=xt[:, ko, :],
                            start=(ko == 0),
                            stop=(ko == KO1 - 1),
                        )
                    nc.scalar.activation(
                        ht[:, m, :], ps1, AF.Gelu, bias=b1_col[:, m : m + 1], scale=1.0
                    )

                ef_blk = [
                    efp.tile([P, H], f32, name=f"ef{tt}", tag="ef")
                    for tt in range(NTT)
                ]
                for tt in range(NTT):
                    for g in range(2):
                        acc = psmm.tile([P, 512], f32, name="acc", tag="mm")
                        for ko in range(KO2):
                            nc.tensor.matmul(
                                acc,
                                lhsT=ht[:, ko, tt * P : (tt + 1) * P],
                                rhs=w2r[ko][:, g * 512 : (g + 1) * 512],
                                start=(ko == 0),
                                stop=(ko == KO2 - 1),
                            )
                        nc.vector.tensor_tensor(
                            ef_blk[tt][:, g * 512 : (g + 1) * 512],
                            acc,
                            b2_bc[:, g * 512 : (g + 1) * 512],
                            ALU.add,
                        )

                for tt in range(NTT):
                    ef_t = ef_blk[tt]
                    stats = lnp.tile([P, 2, 6], f32, name="stats", tag="stats")
                    for sg in range(2):
                        nc.vector.bn_stats(
                            stats[:, sg, :], ef_t[:, sg * 512 : (sg + 1) * 512]
                        )
                    mv = lnp.tile([P, 2], f32, name="mv", tag="mv")
                    nc.vector.bn_aggr(mv, stats)
                    nc.scalar.activation(
                        mv[:, 1:2], mv[:, 1:2], AF.Sqrt, bias=eps_t, scale=1.0
                    )
                    nc.vector.reciprocal(mv[:, 1:2], mv[:, 1:2])
                    nc.vector.tensor_scalar(
                        ef_t,
                        ef_t,
                        scalar1=mv[:, 0:1],
                        scalar2=mv[:, 1:2],
                        op0=ALU.subtract,
                        op1=ALU.mult,
                    )
                    nc.vector.tensor_tensor(ef_t, ef_t, g_bc, ALU.mult)
                    nc.vector.tensor_tensor(ef_t, ef_t, beta_bc, ALU.add)
                    nc.sync.dma_start(
                        out=ef_out[b * TB + tt * P : b * TB + (tt + 1) * P, :],
                        in_=ef_t,
                    )


        # ---------- post: reasoning + validation ----------
        with (
            tc.tile_pool(name="rows", bufs=3) as rows,
            tc.tile_pool(name="scal", bufs=12) as scal_p,
            tc.tile_pool(name="entp", bufs=1) as entp,
            tc.tile_pool(name="rewp2", bufs=3) as rewp2,
            tc.tile_pool(name="etsp2", bufs=3) as etsp2,
            tc.tile_pool(name="rnwp", bufs=6) as rnwp,
            tc.tile_pool(name="vnwp", bufs=2) as vnwp,
            tc.tile_pool(name="psb_va", bufs=6, space="PSUM") as psb_va,
            tc.tile_pool(name="psb_ca", bufs=2, space="PSUM") as psb_ca,
        ):
            # run the relation/sims chain with post-A pools
            rewp = rewp2
            etsp = etsp2
            psva = psb_va
            psca = psb_ca
            seg_re1()
            seg_re2()
            seg_sims()

            # --- gather retrieved entities (resident table, exact fp32) ---
            onehot = state["onehot"]
            ent_nat = []
            for et in range(8):
                rows_e = P if et < 7 else NE - 7 * P
                t_ = entp.tile([P, H], f32, name=f"ent{et}", tag=f"ent{et}")
                nc.sync.dma_start(
                    out=t_[:rows_e, :], in_=ent[et * P : et * P + rows_e, :]
                )
                ent_nat.append((t_, rows_e))
            ret_col = colp.tile([P, KO1, TOPK], f32, name="ret_col", tag="ret_col")
            for fo in range(KO1):
                ps_g = psb_ca.tile([P, TOPK], f32, name="ps_g", tag="colacc")
                for et in range(8):
                    t_, rows_e = ent_nat[et]
                    nc.tensor.matmul(
                        ps_g,
                        lhsT=t_[:rows_e, fo * P : (fo + 1) * P],
                        rhs=onehot[:rows_e, et, :],
                        start=(et == 0),
                        stop=(et == 7),
                    )
                nc.vector.tensor_copy(ret_col[:, fo, :], ps_g)
            ret_row = rows.tile([TOPK, H], f32, name="ret_row", tag="rows", bufs=6)
            for fo in range(KO1):
                ps_r5 = psb_ca.tile([TOPK, P], f32, name="ps_r5", tag="colacc")
                nc.tensor.transpose(ps_r5, ret_col[:, fo, :], ident)
                nc.vector.tensor_copy(ret_row[:, fo * P : (fo + 1) * P], ps_r5)
            nc.sync.dma_start(out=ret_out[:, :], in_=ret_row)
            ev_col = colp.tile([P, KO1], f32, name="ev_col", tag="ev_col")
            nc.vector.tensor_reduce(
                ev_col, ret_col, axis=mybir.AxisListType.X, op=ALU.add
            )
            nc.scalar.mul(ev_col, ev_col, 1.0 / TOPK)
            rin_col = colp.tile([P, 24], f32r, name="rin_col", tag="rin_col")
            nc.vector.tensor_copy(rin_col[:, 0:8], ev_col)
            nc.vector.tensor_copy(rin_col[:, 8:16], state["rf_col"])
            nc.vector.tensor_copy(rin_col[:, 16:24], pooled_col)

            def row_to_col2(row, n_chunks, nm):
                ps_col = psb_ca.tile([P, 16], f32, name=f"{nm}_ps", tag="colacc")
                for ko in range(n_chunks):
                    nc.tensor.matmul(
                        ps_col[:, ko : ko + 1],
                        lhsT=row[:, ko * P : (ko + 1) * P],
                        rhs=one1,
                        start=True,
                        stop=True,
                    )
                col = colp.tile([P, n_chunks], f32, name=f"{nm}_col", tag=f"{nm}_col")
                nc.vector.tensor_copy(col, ps_col[:, :n_chunks])
                return col

            rnb1 = load_row(rn_b1, H2, "rnb1")
            h1r_row = row_tile(H2, "h1r_row")
            ps_rn = [
                psb_va.tile([1, 512], f32, name=f"ps_rn{n}", tag="vecacc")
                for n in range(4)
            ]
            for ko in range(24):
                rwr = rnwp.tile([P, H2], f32r, name="rnw", tag="rnw")
                nc.sync.dma_start(
                    out=rwr, in_=rn_w1[ko * P : (ko + 1) * P, :].bitcast(f32r)
                )
                for n in range(4):
                    nc.tensor.matmul(
                        ps_rn[n],
                        lhsT=rin_col[:, ko : ko + 1],
                        rhs=rwr[:, n * 512 : (n + 1) * 512],
                        start=(ko == 0),
                        stop=(ko == 23),
                    )
            for n in range(4):
                nc.vector.tensor_tensor(
                    h1r_row[:, n * 512 : (n + 1) * 512],
                    ps_rn[n],
                    rnb1[:, n * 512 : (n + 1) * 512],
                    ALU.add,
                )
            nc.scalar.activation(h1r_row, h1r_row, AF.Gelu)
            g1r_col = row_to_col2(h1r_row, 16, "g1r")
            g1r_r = colp.tile([P, 16], f32r, name="g1r_r", tag="g1r_r")
            nc.vector.tensor_copy(g1r_r, g1r_col)

            rnb2 = load_row(rn_b2, H, "rnb2")
            r2_row = row_tile(H, "r2_row")
            ps_rn2 = [
                psb_va.tile([1, 512], f32, name=f"ps_rn2{n}", tag="vecacc")
                for n in range(2)
            ]
            for ko in range(16):
                rwr = rnwp.tile([P, H], f32r, name="rnw2", tag="rnw")
                nc.sync.dma_start(
                    out=rwr, in_=rn_w2[ko * P : (ko + 1) * P, :].bitcast(f32r)
                )
                for n in range(2):
                    nc.tensor.matmul(
                        ps_rn2[n],
                        lhsT=g1r_r[:, ko : ko + 1],
                        rhs=rwr[:, n * 512 : (n + 1) * 512],
                        start=(ko == 0),
                        stop=(ko == 15),
                    )
            for n in range(2):
                nc.vector.tensor_tensor(
                    r2_row[:, n * 512 : (n + 1) * 512],
                    ps_rn2[n],
                    rnb2[:, n * 512 : (n + 1) * 512],
                    ALU.add,
                )
            rng_row = load_row(rn_g, H, "rng_row")
            rnbeta_row = load_row(rn_beta, H, "rnbeta_row")
            ro_row = _row_layernorm(
                nc, rows, scal_p, r2_row, rng_row, rnbeta_row, eps1, "ro"
            )
            nc.sync.dma_start(out=ro_out[:, :], in_=ro_row)

            ro_col = row_to_col2(ro_row, KO1, "roc")
            vnb1_row = load_row(vn_b1, H // 2, "vnb1_row")
            ps_v = psb_va.tile([1, 512], f32, name="ps_v", tag="vecacc")
            for ko in range(KO1):
                vw = vnwp.tile([P, 512], f32, name="vnw", tag="vnw")
                nc.sync.dma_start(out=vw, in_=vn_w1[ko * P : (ko + 1) * P, :])
                nc.tensor.matmul(
                    ps_v,
                    lhsT=ro_col[:, ko : ko + 1],
                    rhs=vw,
                    start=(ko == 0),
                    stop=(ko == KO1 - 1),
                )
            v1_row = row_tile(512, "v1_row")
            nc.vector.tensor_tensor(v1_row, ps_v, vnb1_row, ALU.add)
            nc.scalar.activation(v1_row, v1_row, AF.Gelu)
            v1_col = row_to_col2(v1_row, 4, "v1")
            vnw2 = colp.tile([P, 4, 1], f32, name="vnw2", tag="vnw2")
            nc.sync.dma_start(
                out=vnw2, in_=vn_w2.rearrange("(ko p) n -> p ko n", p=P)
            )
            vnb2_row = scal_p.tile([1, 1], f32, name="vnb2_row", tag="scal")
            nc.sync.dma_start(out=vnb2_row, in_=vn_b2[:, :])
            ps_v2 = psb_va.tile([1, 1], f32, name="ps_v2", tag="vecacc")
            for ko in range(4):
                nc.tensor.matmul(
                    ps_v2,
                    lhsT=v1_col[:, ko : ko + 1],
                    rhs=vnw2[:, ko, :],
                    start=(ko == 0),
                    stop=(ko == 3),
                )
            v2_row = scal_p.tile([1, 1], f32, name="v2_row", tag="scal")
            nc.vector.tensor_tensor(v2_row, ps_v2, vnb2_row, ALU.add)
            nc.scalar.activation(v2_row, v2_row, AF.Sigmoid)
            nc.sync.dma_start(out=vs_out[:, :], in_=v2_row)

    nc.compile()
    return nc


_NC_CACHE = None


def _get_nc():
    global _NC_CACHE
    if _NC_CACHE is None:
        _NC_CACHE = build_kernel()
    return _NC_CACHE


def run(inputs, trace=False):
    from concourse.bass_utils import run_bass_kernel_spmd

    nc = _get_nc()
    hs = np.ascontiguousarray(inputs["hidden_states"], dtype=np.float32)
    B = hs.shape[0]
    assert B == N_CORES

    def row(name, n):
        return np.ascontiguousarray(
            np.asarray(inputs[name], dtype=np.float32).reshape(1, n)
        )

    E = np.ascontiguousarray(inputs["entity_embeddings"], np.float32)
    shared = {
        "ent": E,
        "entT": np.ascontiguousarray(E.T),
        "ee_w1": np.ascontiguousarray(inputs["ee_w1"], np.float32),
        "ee_b1": row("ee_b1", H2),
        "ee_w2": np.ascontiguousarray(inputs["ee_w2"], np.float32),
        "ee_b2": row("ee_b2", H),
        "ee_g": row("ee_g", H),
        "ee_beta": row("ee_beta", H),
        "re_w1": np.ascontiguousarray(inputs["re_w1"], np.float32),
        "re_b1": row("re_b1", H),
        "re_w2": np.ascontiguousarray(inputs["re_w2"], np.float32),
        "re_b2": row("re_b2", H),
        "re_g": row("re_g", H),
        "re_beta": row("re_beta", H),
        "rn_w1": np.ascontiguousarray(inputs["rn_w1"], np.float32),
        "rn_b1": row("rn_b1", H2),
        "rn_w2": np.ascontiguousarray(inputs["rn_w2"], np.float32),
        "rn_b2": row("rn_b2", H),
        "rn_g": row("rn_g", H),
        "rn_beta": row("rn_beta", H),
        "vn_w1": np.ascontiguousarray(inputs["vn_w1"], np.float32),
        "vn_b1": row("vn_b1", H // 2),
        "vn_w2": np.ascontiguousarray(inputs["vn_w2"], np.float32),
        "vn_b2": row("vn_b2", 1),
    }
    in_maps = [
        dict(shared, xT=np.ascontiguousarray(hs[c].T)) for c in range(B)
    ]
    res = run_bass_kernel_spmd(
        nc, in_maps, core_ids=list(range(N_CORES)), trace=trace
    )
    r = res.results
    ef = np.stack([r[c]["ef_out"] for c in range(B)])
    rf = np.stack([r[c]["rf_out"][0] for c in range(B)])
    ret = np.stack([r[c]["ret_out"] for c in range(B)])
    sims = np.stack([r[c]["sims_out"][0] for c in range(B)])
    idx = np.stack([r[c]["idx_out"][0] for c in range(B)]).astype(np.int32)
    ro = np.stack([r[c]["ro_out"][0] for c in range(B)])
    vs = np.stack([r[c]["vs_out"][0] for c in range(B)])
    out = (ef, rf, ret, sims, idx, ro, vs)
    return (out, res) if trace else out


def kernel(**inputs):
    return run(inputs, trace=False)


# revision 31
# speedup vs baseline: 1.4122x; 1.1103x over previous
"""Trainium2 Bass kernel for nn_KnowledgeBaseModule.

Data-parallel over batch: 8 batch rows -> 8 NeuronCores, weights/tables
replicated. Entity encoder (the dominant compute) runs in feature-major
(transposed) layout so both weight matmuls use natural weight layouts; the
two big matmuls use float32r (fast fp32) operands, everything index-critical
stays fp32.
"""

import numpy as np

import concourse.bass as bass
import concourse.mybir as mybir
import concourse.tile as tile
from concourse import bacc
from concourse.masks import make_identity

f32 = mybir.dt.float32
f32r = mybir.dt.float32r
i32 = mybir.dt.int32
u32 = mybir.dt.uint32

P = 128
H = 1024
H2 = 2048
T = 2048          # tokens per core (one batch row)
TB = 512          # token block
NBLK = T // TB    # 4
NE = 1000
TOPK = 5
EPS = 1e-5
KO1 = H // P      # 8
MO1 = H2 // P     # 16
KO2 = H2 // P     # 16
MO2 = H // P      # 8
N_CORES = 8

AF = mybir.ActivationFunctionType
ALU = mybir.AluOpType


def _row_to_col(nc, ps_pool, one1, col_pool, row, n_chunks, name):
    """[1, n_chunks*128] row -> [128, n_chunks] column layout via K=1 matmuls."""
    ps_col = ps_pool.tile([P, n_chunks], f32, name=f"{name}_ps", tag="colacc")
    for ko in range(n_chunks):
        nc.tensor.matmul(
            ps_col[:, ko : ko + 1],
            lhsT=row[:, ko * P : (ko + 1) * P],
            rhs=one1,
            start=True,
            stop=True,
        )
    col = col_pool.tile([P, n_chunks], f32, name=f"{name}_col", tag=f"{name}_col")
    nc.vector.tensor_copy(col, ps_col)
    return col


def _row_layernorm(nc, rows_pool, scal_pool, row, g_row, b_row, eps1, name):
    """LayerNorm along the free dim of a [1, N] row (single partition)."""
    n = row.shape[1]

    def scal(nm):
        return scal_pool.tile([1, 1], f32, name=f"{name}_{nm}", tag="scal")

    s1 = scal("s1")
    s2 = scal("s2")
    sq = rows_pool.tile([1, n], f32, name=f"{name}_sq", tag="rows", bufs=6)
    nc.vector.tensor_reduce(s1, row, axis=mybir.AxisListType.X, op=ALU.add)
    nc.vector.tensor_tensor(sq, row, row, ALU.mult)
    nc.vector.tensor_reduce(s2, sq, axis=mybir.AxisListType.X, op=ALU.add)
    mean = scal("mean")
    msq = scal("msq")
    nc.scalar.mul(mean, s1, 1.0 / n)
    nc.scalar.mul(msq, s2, 1.0 / n)
    m2 = scal("m2")
    var = scal("var")
    nc.vector.tensor_tensor(m2, mean, mean, ALU.mult)
    nc.vector.tensor_tensor(var, msq, m2, ALU.subtract)
    nc.scalar.activation(var, var, AF.Sqrt, bias=eps1, scale=1.0)
    nc.vector.reciprocal(var, var)
    out = rows_pool.tile([1, n], f32, name=f"{name}_out", tag="rows", bufs=6)
    nc.vector.tensor_scalar(
        out, row, scalar1=mean, scalar2=var, op0=ALU.subtract, op1=ALU.mult
    )
    nc.vector.tensor_tensor(out, out, g_row, ALU.mult)
    nc.vector.tensor_tensor(out, out, b_row, ALU.add)
    return out


def build_kernel():
    import concourse.tile_utils as tile_utils

    tile_utils.max_sbuf_usage = 208 * 1024

    nc = bacc.Bacc()

    # ---- DRAM I/O ----
    xT = nc.dram_tensor("xT", [H, T], f32, kind="ExternalInput")
    ent = nc.dram_tensor("ent", [NE, H], f32, kind="ExternalInput")
    entT = nc.dram_tensor("entT", [H, NE], f32, kind="ExternalInput")
    ee_w1 = nc.dram_tensor("ee_w1", [H, H2], f32, kind="ExternalInput")
    ee_b1 = nc.dram_tensor("ee_b1", [1, H2], f32, kind="ExternalInput")
    ee_w2 = nc.dram_tensor("ee_w2", [H2, H], f32, kind="ExternalInput")
    ee_b2 = nc.dram_tensor("ee_b2", [1, H], f32, kind="ExternalInput")
    ee_g = nc.dram_tensor("ee_g", [1, H], f32, kind="ExternalInput")
    ee_beta = nc.dram_tensor("ee_beta", [1, H], f32, kind="ExternalInput")
    re_w1 = nc.dram_tensor("re_w1", [H, H], f32, kind="ExternalInput")
    re_b1 = nc.dram_tensor("re_b1", [1, H], f32, kind="ExternalInput")
    re_w2 = nc.dram_tensor("re_w2", [H, H], f32, kind="ExternalInput")
    re_b2 = nc.dram_tensor("re_b2", [1, H], f32, kind="ExternalInput")
    re_g = nc.dram_tensor("re_g", [1, H], f32, kind="ExternalInput")
    re_beta = nc.dram_tensor("re_beta", [1, H], f32, kind="ExternalInput")
    rn_w1 = nc.dram_tensor("rn_w1", [3 * H, H2], f32, kind="ExternalInput")
    rn_b1 = nc.dram_tensor("rn_b1", [1, H2], f32, kind="ExternalInput")
    rn_w2 = nc.dram_tensor("rn_w2", [H2, H], f32, kind="ExternalInput")
    rn_b2 = nc.dram_tensor("rn_b2", [1, H], f32, kind="ExternalInput")
    rn_g = nc.dram_tensor("rn_g", [1, H], f32, kind="ExternalInput")
    rn_beta = nc.dram_tensor("rn_beta", [1, H], f32, kind="ExternalInput")
    vn_w1 = nc.dram_tensor("vn_w1", [H, H // 2], f32, kind="ExternalInput")
    vn_b1 = nc.dram_tensor("vn_b1", [1, H // 2], f32, kind="ExternalInput")
    vn_w2 = nc.dram_tensor("vn_w2", [H // 2, 1], f32, kind="ExternalInput")
    vn_b2 = nc.dram_tensor("vn_b2", [1, 1], f32, kind="ExternalInput")

    ef_out = nc.dram_tensor("ef_out", [T, H], f32, kind="ExternalOutput")
    rf_out = nc.dram_tensor("rf_out", [1, H], f32, kind="ExternalOutput")
    ret_out = nc.dram_tensor("ret_out", [TOPK, H], f32, kind="ExternalOutput")
    sims_out = nc.dram_tensor("sims_out", [1, NE], f32, kind="ExternalOutput")
    idx_out = nc.dram_tensor("idx_out", [1, TOPK], i32, kind="ExternalOutput")
    ro_out = nc.dram_tensor("ro_out", [1, H], f32, kind="ExternalOutput")
    vs_out = nc.dram_tensor("vs_out", [1, 1], f32, kind="ExternalOutput")

    from contextlib import ExitStack

    with tile.TileContext(nc) as tc, ExitStack() as stack:
        # ---------- persistent pools ----------
        cpool = stack.enter_context(tc.tile_pool(name="consts", bufs=1))
        colp = stack.enter_context(tc.tile_pool(name="colp", bufs=1))
        rows = None
        scal_p = None

        ident = cpool.tile([P, P], f32)
        make_identity(nc, ident)
        b1_col = cpool.tile([P, MO1], f32)
        nc.sync.dma_start(out=b1_col, in_=ee_b1.rearrange("a (m p) -> p (a m)", p=P))
        b2_bc = cpool.tile([P, H], f32)
        nc.sync.dma_start(out=b2_bc, in_=ee_b2[:, :].to_broadcast([P, H]))
        g_bc = cpool.tile([P, H], f32)
        nc.sync.dma_start(out=g_bc, in_=ee_g[:, :].to_broadcast([P, H]))
        beta_bc = cpool.tile([P, H], f32)
        nc.sync.dma_start(out=beta_bc, in_=ee_beta[:, :].to_broadcast([P, H]))
        eps_t = cpool.tile([P, 1], f32)
        nc.vector.memset(eps_t, EPS)
        eps1 = cpool.tile([1, 1], f32)
        nc.vector.memset(eps1, EPS)
        ones_row = cpool.tile([1, P], f32)
        nc.vector.memset(ones_row, 1.0)
        one1 = cpool.tile([1, 1], f32)
        nc.vector.memset(one1, 1.0)
        pooled_acc = cpool.tile([P, KO1], f32)
        nc.vector.memset(pooled_acc, 0.0)
        rin_col = colp.tile([P, 24], f32r, name="rin_col", tag="rin_col")

        def row_tile(n, nm):
            if n > 1024:
                return rows.tile([1, n], f32, name=nm, tag="rows2", bufs=2)
            return rows.tile([1, n], f32, name=nm, tag="rows", bufs=6)

        def load_row(dram, n, nm):
            t_ = row_tile(n, nm)
            nc.sync.dma_start(out=t_, in_=dram[:, :])
            return t_

        # ---------- fused phase A + interleaved vector chain ----------
        with (
            tc.tile_pool(name="w1p", bufs=1) as w1p,
            tc.tile_pool(name="w2p", bufs=1) as w2p,
            tc.tile_pool(name="xtp", bufs=1) as xtp,
            tc.tile_pool(name="htp", bufs=1) as htp,
            tc.tile_pool(name="efp", bufs=4) as efp,
            tc.tile_pool(name="lnp", bufs=4) as lnp,
            tc.tile_pool(name="psmm", bufs=8, space="PSUM") as psmm,
        ):
            NTT = TB // P

            # block-0 activations first so the PE can start ASAP
            xt0 = xtp.tile([P, KO1, TB], f32r, name="xt", tag="xt")
            nc.sync.dma_start(
                out=xt0,
                in_=xT[:, 0:TB].rearrange("(ko p) t -> p ko t", p=P).bitcast(f32r),
            )

            # --- W1 + W2 resident as f32r ---
            w1r = []
            for ko in range(KO1):
                wr = w1p.tile([P, H2], f32r, name=f"w1r{ko}", tag=f"w1r{ko}")
                nc.sync.dma_start(
                    out=wr, in_=ee_w1[ko * P : (ko + 1) * P, :].bitcast(f32r)
                )
                w1r.append(wr)
            w2r = []
            for ko in range(KO2):
                wr = w2p.tile([P, H], f32r, name=f"w2r{ko}", tag=f"w2r{ko}")
                nc.sync.dma_start(
                    out=wr, in_=ee_w2[ko * P : (ko + 1) * P, :].bitcast(f32r)
                )
                w2r.append(wr)

            # --- chain segment builders (emitted between blocks) ---
            def vec_mlp_layer(w_dram, in_col, n_out, bias_row, nm):
                """[1, n_out] = in_col.T @ W + bias, fp32, weights streamed."""
                out_row = row_tile(n_out, f"{nm}_row")
                nko = in_col.shape[1]
                for n in range((n_out + 511) // 512):
                    nsz = min(512, n_out - n * 512)
                    ps_a = psva.tile([1, 512], f32, name=f"{nm}ps{n}", tag="vecacc")
                    for ko in range(nko):
                        wch = rewp.tile([P, 512], f32, name=f"{nm}w", tag="rew")
                        nc.sync.dma_start(
                            out=wch[:, :nsz],
                            in_=w_dram[
                                ko * P : (ko + 1) * P, n * 512 : n * 512 + nsz
                            ],
                        )
                        nc.tensor.matmul(
                            ps_a[:, :nsz],
                            lhsT=in_col[:, ko : ko + 1],
                            rhs=wch[:, :nsz],
                            start=(ko == 0),
                            stop=(ko == nko - 1),
                        )
                    nc.vector.tensor_tensor(
                        out_row[:, n * 512 : n * 512 + nsz],
                        ps_a[:, :nsz],
                        bias_row[:, n * 512 : n * 512 + nsz],
                        ALU.add,
                    )
                return out_row

            def row_to_col(row, n_chunks, nm):
                ps_col = psca.tile([P, 16], f32, name=f"{nm}_ps", tag="colacc")
                for ko in range(n_chunks):
                    nc.tensor.matmul(
                        ps_col[:, ko : ko + 1],
                        lhsT=row[:, ko * P : (ko + 1) * P],
                        rhs=one1,
                        start=True,
                        stop=True,
                    )
                col = colp.tile([P, n_chunks], f32, name=f"{nm}_col", tag=f"{nm}_col")
                nc.vector.tensor_copy(col, ps_col[:, :n_chunks])
                return col

            state = {}

            def seg_re1():
                reb1 = load_row(re_b1, H, "reb1")
                h1 = vec_mlp_layer(re_w1, pooled_col, H, reb1, "re1")
                nc.scalar.activation(h1, h1, AF.Gelu)
                state["g1_col"] = row_to_col(h1, KO1, "g1")

            def seg_re2():
                reb2 = load_row(re_b2, H, "reb2")
                r_row = vec_mlp_layer(re_w2, state["g1_col"], H, reb2, "re2")
                reg_row = load_row(re_g, H, "reg")
                rebeta_row = load_row(re_beta, H, "rebeta")
                rf_row = _row_layernorm(
                    nc, rows, scal_p, r_row, reg_row, rebeta_row, eps1, "rf"
                )
                nc.sync.dma_start(out=rf_out[:, :], in_=rf_row)
                state["rf_col"] = row_to_col(rf_row, KO1, "rf")
                nc.vector.tensor_copy(rin_col[:, 8:16], state["rf_col"])

            def seg_sims():
                rf_col = state["rf_col"]
                sims_row = row_tile(NE, "sims_row")
                for n in range(2):
                    nsz = 512 if n == 0 else NE - 512
                    ps_s = psva.tile([1, 512], f32, name=f"simsps{n}", tag="vecacc")
                    for ko in range(KO1):
                        ech = etsp.tile([P, 512], f32, name="ech", tag="ets")
                        nc.sync.dma_start(
                            out=ech[:, :nsz],
                            in_=entT[
                                ko * P : (ko + 1) * P, n * 512 : n * 512 + nsz
                            ],
                        )
                        nc.tensor.matmul(
                            ps_s[:, :nsz],
                            lhsT=rf_col[:, ko : ko + 1],
                            rhs=ech[:, :nsz],
                            start=(ko == 0),
                            stop=(ko == KO1 - 1),
                        )
                    nc.vector.tensor_copy(
                        sims_row[:, n * 512 : n * 512 + nsz], ps_s[:, :nsz]
                    )
                nc.sync.dma_start(out=sims_out[:, :], in_=sims_row)
                mxv = colp.tile([1, 8], f32, name="mxv", tag="mxv")
                mxi = colp.tile([1, 8], u32, name="mxi", tag="mxi")
                nc.vector.max_with_indices(mxv, mxi, sims_row)
                mii = colp.tile([1, 8], i32, name="mii", tag="mii")
                nc.vector.tensor_copy(mii, mxi)
                nc.sync.dma_start(out=idx_out[:, :], in_=mii[:, :TOPK])
                mif = colp.tile([1, 8], f32, name="mif", tag="mif")
                nc.vector.tensor_copy(mif, mxi)
                ps_ib = psca.tile([P, 8], f32, name="ps_ib", tag="colacc")
                nc.tensor.matmul(
                    ps_ib, lhsT=ones_row, rhs=mif, start=True, stop=True
                )
                idx_b = colp.tile([P, 8], f32, name="idx_b", tag="idx_b")
                nc.vector.tensor_copy(idx_b, ps_ib)
                iota_i = colp.tile([P, 8], i32, name="iota_i", tag="iota_i")
                nc.gpsimd.iota(
                    iota_i, pattern=[[P, 8]], base=0, channel_multiplier=1
                )
                iota_f = colp.tile([P, 8], f32, name="iota_f", tag="iota_f")
                nc.vector.tensor_copy(iota_f, iota_i)
                onehot = colp.tile([P, 8, TOPK], f32, name="onehot", tag="onehot")
                for et in range(8):
                    nc.vector.tensor_tensor(
                        onehot[:, et, :],
                        idx_b[:, :TOPK],
                        iota_f[:, et : et + 1].to_broadcast([P, TOPK]),
                        ALU.is_equal,
                    )
                state["onehot"] = onehot

            segments = [seg_re1, seg_re2, seg_sims]

            # --- phase A blocks with interleaved segments ---
            pooled_col = colp.tile([P, KO1], f32, name="pooled_col", tag="pooled_col")
            for b in range(NBLK):
                if b == 0:
                    xt = xt0
                else:
                    xt = xtp.tile([P, KO1, TB], f32r, name="xt", tag="xt")
                    nc.sync.dma_start(
                        out=xt,
                        in_=xT[:, b * TB : (b + 1) * TB]
                        .rearrange("(ko p) t -> p ko t", p=P)
                        .bitcast(f32r),
                    )
                # pooled: exact f32 view, 2-stage reduce for short sum chains
                red2 = lnp.tile([P, KO1, 4], f32, name="red2", tag="red2")
                nc.vector.tensor_reduce(
                    red2,
                    xt.bitcast(f32).rearrange("p ko (c t) -> p ko c t", c=4),
                    axis=mybir.AxisListType.X,
                    op=ALU.add,
                )
                red = lnp.tile([P, KO1], f32, name="red", tag="red")
                nc.vector.tensor_reduce(
                    red, red2, axis=mybir.AxisListType.X, op=ALU.add
                )
                nc.vector.tensor_tensor(pooled_acc, pooled_acc, red, ALU.add)
                if b == NBLK - 1:
                    nc.scalar.mul(pooled_col, pooled_acc, 1.0 / T)
                    nc.vector.tensor_copy(rin_col[:, 16:24], pooled_col)

                ht = htp.tile([P, MO1, TB], f32r, name="ht", tag="ht")
                for m in range(MO1):
                    ps1 = psmm.tile([P, TB], f32, name="ps1", tag="mm")
                    for ko in range(KO1):
                        nc.tensor.matmul(
                            ps1,
                            lhsT=w1r[ko][:, m * P : (m + 1) * P],
                            rhs=xt[:, ko, :],
                            start=(ko == 0),
                            stop=(ko == KO1 - 1),
                        )
                    nc.scalar.activation(
                        ht[:, m, :], ps1, AF.Gelu, bias=b1_col[:, m : m + 1], scale=1.0
                    )

                ef_blk = [
                    efp.tile([P, H], f32, name=f"ef{tt}", tag="ef")
                    for tt in range(NTT)
                ]
                for tt in range(NTT):
                    for g in range(2):
                        acc = psmm.tile([P, 512], f32, name="acc", tag="mm")
                        for ko in range(KO2):
                            nc.tensor.matmul(
                                acc,
                                lhsT=ht[:, ko, tt * P : (tt + 1) * P],
                                rhs=w2r[ko][:, g * 512 : (g + 1) * 512],
                                start=(ko == 0),
                                stop=(ko == KO2 - 1),
                            )
                        nc.vector.tensor_tensor(
                            ef_blk[tt][:, g * 512 : (g + 1) * 512],
                            acc,
                            b2_bc[:, g * 512 : (g + 1) * 512],
                            ALU.add,
                        )

                for tt in range(NTT):
                    ef_t = ef_blk[tt]
                    stats = lnp.tile([P, 2, 6], f32, name="stats", tag="stats")
                    for sg in range(2):
                        nc.vector.bn_stats(
                            stats[:, sg, :], ef_t[:, sg * 512 : (sg + 1) * 512]
                        )
                    mv = lnp.tile([P, 2], f32, name="mv", tag="mv")
                    nc.vector.bn_aggr(mv, stats)
                    nc.scalar.activation(
                        mv[:, 1:2], mv[:, 1:2], AF.Sqrt, bias=eps_t, scale=1.0
                    )
                    nc.vector.reciprocal(mv[:, 1:2], mv[:, 1:2])
                    nc.vector.tensor_scalar(
                        ef_t,
                        ef_t,
                        scalar1=mv[:, 0:1],
                        scalar2=mv[:, 1:2],
                        op0=ALU.subtract,
                        op1=ALU.mult,
                    )
                    nc.vector.tensor_tensor(ef_t, ef_t, g_bc, ALU.mult)
                    nc.vector.tensor_tensor(ef_t, ef_t, beta_bc, ALU.add)
                    nc.sync.dma_start(
                        out=ef_out[b * TB + tt * P : b * TB + (tt + 1) * P, :],
                        in_=ef_t,
                    )


        # ---------- post: reasoning + validation ----------
        with (
            tc.tile_pool(name="rows", bufs=3) as rows,
            tc.tile_pool(name="scal", bufs=12) as scal_p,
            tc.tile_pool(name="entp", bufs=1) as entp,
            tc.tile_pool(name="rewp2", bufs=3) as rewp2,
            tc.tile_pool(name="etsp2", bufs=3) as etsp2,
            tc.tile_pool(name="rnwp", bufs=6) as rnwp,
            tc.tile_pool(name="vnwp", bufs=2) as vnwp,
            tc.tile_pool(name="psb_va", bufs=6, space="PSUM") as psb_va,
            tc.tile_pool(name="psb_ca", bufs=2, space="PSUM") as psb_ca,
        ):
            # run the relation/sims chain with post-A pools
            rewp = rewp2
            etsp = etsp2
            psva = psb_va
            psca = psb_ca
            seg_re1()
            seg_re2()
            seg_sims()

            # --- gather retrieved entities (resident table, exact fp32) ---
            onehot = state["onehot"]
            ent_nat = []
            for et in range(8):
                rows_e = P if et < 7 else NE - 7 * P
                t_ = entp.tile([P, H], f32, name=f"ent{et}", tag=f"ent{et}")
                nc.sync.dma_start(
                    out=t_[:rows_e, :], in_=ent[et * P : et * P + rows_e, :]
                )
                ent_nat.append((t_, rows_e))
            ret_col = colp.tile([P, KO1, TOPK], f32, name="ret_col", tag="ret_col")
            for fo in range(KO1):
                ps_g = psb_ca.tile([P, TOPK], f32, name="ps_g", tag="colacc")
                for et in range(8):
                    t_, rows_e = ent_nat[et]
                    nc.tensor.matmul(
                        ps_g,
                        lhsT=t_[:rows_e, fo * P : (fo + 1) * P],
                        rhs=onehot[:rows_e, et, :],
                        start=(et == 0),
                        stop=(et == 7),
                    )
                nc.vector.tensor_copy(ret_col[:, fo, :], ps_g)
            ret_row = rows.tile([TOPK, H], f32, name="ret_row", tag="rows", bufs=6)
            for fo in range(KO1):
                ps_r5 = psb_ca.tile([TOPK, P], f32, name="ps_r5", tag="colacc")
                nc.tensor.transpose(ps_r5, ret_col[:, fo, :], ident)
                nc.vector.tensor_copy(ret_row[:, fo * P : (fo + 1) * P], ps_r5)
            nc.sync.dma_start(out=ret_out[:, :], in_=ret_row)
            ev_col = colp.tile([P, KO1], f32, name="ev_col", tag="ev_col")
            nc.vector.tensor_reduce(
                ev_col, ret_col, axis=mybir.AxisListType.X, op=ALU.add
            )
            nc.scalar.mul(ev_col, ev_col, 1.0 / TOPK)
            nc.vector.tensor_copy(rin_col[:, 0:8], ev_col)

            def row_to_col2(row, n_chunks, nm):
                ps_col = psb_ca.tile([P, 16], f32, name=f"{nm}_ps", tag="colacc")
                for ko in range(n_chunks):
                    nc.tensor.matmul(
                        ps_col[:, ko : ko + 1],
                        lhsT=row[:, ko * P : (ko + 1) * P],
                        rhs=one1,
                        start=True,
                        stop=True,
                    )
                col = colp.tile([P, n_chunks], f32, name=f"{nm}_col", tag=f"{nm}_col")
                nc.vector.tensor_copy(col, ps_col[:, :n_chunks])
                return col

            rnb1 = load_row(rn_b1, H2, "rnb1")
            h1r_row = row_tile(H2, "h1r_row")
            ps_rn = [
                psb_va.tile([1, 512], f32, name=f"ps_rn{n}", tag="vecacc")
                for n in range(4)
            ]
            rn1_order = list(range(16, 24)) + list(range(8, 16)) + list(range(8))
            for i, ko in enumerate(rn1_order):
                rwr = rnwp.tile([P, H2], f32r, name="rnw", tag="rnw")
                nc.sync.dma_start(
                    out=rwr, in_=rn_w1[ko * P : (ko + 1) * P, :].bitcast(f32r)
                )
                for n in range(4):
                    nc.tensor.matmul(
                        ps_rn[n],
                        lhsT=rin_col[:, ko : ko + 1],
                        rhs=rwr[:, n * 512 : (n + 1) * 512],
                        start=(i == 0),
                        stop=(i == 23),
                    )
            for n in range(4):
                nc.vector.tensor_tensor(
                    h1r_row[:, n * 512 : (n + 1) * 512],
                    ps_rn[n],
                    rnb1[:, n * 512 : (n + 1) * 512],
                    ALU.add,
                )
            nc.scalar.activation(h1r_row, h1r_row, AF.Gelu)
            g1r_col = row_to_col2(h1r_row, 16, "g1r")
            g1r_r = colp.tile([P, 16], f32r, name="g1r_r", tag="g1r_r")
            nc.vector.tensor_copy(g1r_r, g1r_col)

            rnb2 = load_row(rn_b2, H, "rnb2")
            r2_row = row_tile(H, "r2_row")
            ps_rn2 = [
                psb_va.tile([1, 512], f32, name=f"ps_rn2{n}", tag="vecacc")
                for n in range(2)
            ]
            for ko in range(16):
                rwr = rnwp.tile([P, H], f32r, name="rnw2", tag="rnw")
                nc.sync.dma_start(
                    out=rwr, in_=rn_w2[ko * P : (ko + 1) * P, :].bitcast(f32r)
                )
                for n in range(2):
                    nc.tensor.matmul(
                        ps_rn2[n],
                        lhsT=g1r_r[:, ko : ko + 1],
                        rhs=rwr[:, n * 512 : (n + 1) * 512],
                        start=(ko == 0),
                        stop=(ko == 15),
                    )
            for n in range(2):
                nc.vector.tensor_tensor(
                    r2_row[:, n * 512 : (n + 1) * 512],
                    ps_rn2[n],
                    rnb2[:, n * 512 : (n + 1) * 512],
                    ALU.add,
                )
            rng_row = load_row(rn_g, H, "rng_row")
            rnbeta_row = load_row(rn_beta, H, "rnbeta_row")
            ro_row = _row_layernorm(
                nc, rows, scal_p, r2_row, rng_row, rnbeta_row, eps1, "ro"
            )
            nc.sync.dma_start(out=ro_out[:, :], in_=ro_row)

            ro_col = row_to_col2(ro_row, KO1, "roc")
            vnb1_row = load_row(vn_b1, H // 2, "vnb1_row")
            ps_v = psb_va.tile([1, 512], f32, name="ps_v", tag="vecacc")
            for ko in range(KO1):
                vw = vnwp.tile([P, 512], f32, name="vnw", tag="vnw")
                nc.sync.dma_start(out=vw, in_=vn_w1[ko * P : (ko + 1) * P, :])
                nc.tensor.matmul(
                    ps_v,
                    lhsT=ro_col[:, ko : ko + 1],
                    rhs=vw,
                    start=(ko == 0),
                    stop=(ko == KO1 - 1),
                )
            v1_row = row_tile(512, "v1_row")
            nc.vector.tensor_tensor(v1_row, ps_v, vnb1_row, ALU.add)
            nc.scalar.activation(v1_row, v1_row, AF.Gelu)
            v1_col = row_to_col2(v1_row, 4, "v1")
            vnw2 = colp.tile([P, 4, 1], f32, name="vnw2", tag="vnw2")
            nc.sync.dma_start(
                out=vnw2, in_=vn_w2.rearrange("(ko p) n -> p ko n", p=P)
            )
            vnb2_row = scal_p.tile([1, 1], f32, name="vnb2_row", tag="scal")
            nc.sync.dma_start(out=vnb2_row, in_=vn_b2[:, :])
            ps_v2 = psb_va.tile([1, 1], f32, name="ps_v2", tag="vecacc")
            for ko in range(4):
                nc.tensor.matmul(
                    ps_v2,
                    lhsT=v1_col[:, ko : ko + 1],
                    rhs=vnw2[:, ko, :],
                    start=(ko == 0),
                    stop=(ko == 3),
                )
            v2_row = scal_p.tile([1, 1], f32, name="v2_row", tag="scal")
            nc.vector.tensor_tensor(v2_row, ps_v2, vnb2_row, ALU.add)
            nc.scalar.activation(v2_row, v2_row, AF.Sigmoid)
            nc.sync.dma_start(out=vs_out[:, :], in_=v2_row)

    nc.compile()
    return nc


_NC_CACHE = None


def _get_nc():
    global _NC_CACHE
    if _NC_CACHE is None:
        _NC_CACHE = build_kernel()
    return _NC_CACHE


def run(inputs, trace=False):
    from concourse.bass_utils import run_bass_kernel_spmd

    nc = _get_nc()
    hs = np.ascontiguousarray(inputs["hidden_states"], dtype=np.float32)
    B = hs.shape[0]
    assert B == N_CORES

    def row(name, n):
        return np.ascontiguousarray(
            np.asarray(inputs[name], dtype=np.float32).reshape(1, n)
        )

    E = np.ascontiguousarray(inputs["entity_embeddings"], np.float32)
    shared = {
        "ent": E,
        "entT": np.ascontiguousarray(E.T),
        "ee_w1": np.ascontiguousarray(inputs["ee_w1"], np.float32),
        "ee_b1": row("ee_b1", H2),
        "ee_w2": np.ascontiguousarray(inputs["ee_w2"], np.float32),
        "ee_b2": row("ee_b2", H),
        "ee_g": row("ee_g", H),
        "ee_beta": row("ee_beta", H),
        "re_w1": np.ascontiguousarray(inputs["re_w1"], np.float32),
        "re_b1": row("re_b1", H),
        "re_w2": np.ascontiguousarray(inputs["re_w2"], np.float32),
        "re_b2": row("re_b2", H),
        "re_g": row("re_g", H),
        "re_beta": row("re_beta", H),
        "rn_w1": np.ascontiguousarray(inputs["rn_w1"], np.float32),
        "rn_b1": row("rn_b1", H2),
        "rn_w2": np.ascontiguousarray(inputs["rn_w2"], np.float32),
        "rn_b2": row("rn_b2", H),
        "rn_g": row("rn_g", H),
        "rn_beta": row("rn_beta", H),
        "vn_w1": np.ascontiguousarray(inputs["vn_w1"], np.float32),
        "vn_b1": row("vn_b1", H // 2),
        "vn_w2": np.ascontiguousarray(inputs["vn_w2"], np.float32),
        "vn_b2": row("vn_b2", 1),
    }
    in_maps = [
        dict(shared, xT=np.ascontiguousarray(hs[c].T)) for c in range(B)
    ]
    res = run_bass_kernel_spmd(
        nc, in_maps, core_ids=list(range(N_CORES)), trace=trace
    )
    r = res.results
    ef = np.stack([r[c]["ef_out"] for c in range(B)])
    rf = np.stack([r[c]["rf_out"][0] for c in range(B)])
    ret = np.stack([r[c]["ret_out"] for c in range(B)])
    sims = np.stack([r[c]["sims_out"][0] for c in range(B)])
    idx = np.stack([r[c]["idx_out"][0] for c in range(B)]).astype(np.int32)
    ro = np.stack([r[c]["ro_out"][0] for c in range(B)])
    vs = np.stack([r[c]["vs_out"][0] for c in range(B)])
    out = (ef, rf, ret, sims, idx, ro, vs)
    return (out, res) if trace else out


def kernel(**inputs):
    return run(inputs, trace=False)


# revision 34
# speedup vs baseline: 1.4188x; 1.0047x over previous
"""Trainium2 Bass kernel for nn_KnowledgeBaseModule.

Data-parallel over batch: 8 batch rows -> 8 NeuronCores, weights/tables
replicated. Entity encoder (the dominant compute) runs in feature-major
(transposed) layout so both weight matmuls use natural weight layouts; the
two big matmuls use float32r (fast fp32) operands, everything index-critical
stays fp32.
"""

import numpy as np

import concourse.bass as bass
import concourse.mybir as mybir
import concourse.tile as tile
from concourse import bacc
from concourse.masks import make_identity

f32 = mybir.dt.float32
f32r = mybir.dt.float32r
i32 = mybir.dt.int32
u32 = mybir.dt.uint32

P = 128
H = 1024
H2 = 2048
T = 2048          # tokens per core (one batch row)
TB = 512          # token block
NBLK = T // TB    # 4
NE = 1000
TOPK = 5
EPS = 1e-5
KO1 = H // P      # 8
MO1 = H2 // P     # 16
KO2 = H2 // P     # 16
MO2 = H // P      # 8
N_CORES = 8

AF = mybir.ActivationFunctionType
ALU = mybir.AluOpType


def _row_to_col(nc, ps_pool, one1, col_pool, row, n_chunks, name):
    """[1, n_chunks*128] row -> [128, n_chunks] column layout via K=1 matmuls."""
    ps_col = ps_pool.tile([P, n_chunks], f32, name=f"{name}_ps", tag="colacc")
    for ko in range(n_chunks):
        nc.tensor.matmul(
            ps_col[:, ko : ko + 1],
            lhsT=row[:, ko * P : (ko + 1) * P],
            rhs=one1,
            start=True,
            stop=True,
        )
    col = col_pool.tile([P, n_chunks], f32, name=f"{name}_col", tag=f"{name}_col")
    nc.vector.tensor_copy(col, ps_col)
    return col


def _row_layernorm(nc, rows_pool, scal_pool, row, g_row, b_row, eps1, name):
    """LayerNorm along the free dim of a [1, N] row (single partition)."""
    n = row.shape[1]

    def scal(nm):
        return scal_pool.tile([1, 1], f32, name=f"{name}_{nm}", tag="scal")

    s1 = scal("s1")
    s2 = scal("s2")
    sq = rows_pool.tile([1, n], f32, name=f"{name}_sq", tag="rows", bufs=6)
    nc.vector.tensor_reduce(s1, row, axis=mybir.AxisListType.X, op=ALU.add)
    nc.vector.tensor_tensor(sq, row, row, ALU.mult)
    nc.vector.tensor_reduce(s2, sq, axis=mybir.AxisListType.X, op=ALU.add)
    mean = scal("mean")
    msq = scal("msq")
    nc.scalar.mul(mean, s1, 1.0 / n)
    nc.scalar.mul(msq, s2, 1.0 / n)
    m2 = scal("m2")
    var = scal("var")
    nc.vector.tensor_tensor(m2, mean, mean, ALU.mult)
    nc.vector.tensor_tensor(var, msq, m2, ALU.subtract)
    nc.scalar.activation(var, var, AF.Sqrt, bias=eps1, scale=1.0)
    nc.vector.reciprocal(var, var)
    out = rows_pool.tile([1, n], f32, name=f"{name}_out", tag="rows", bufs=6)
    nc.vector.tensor_scalar(
        out, row, scalar1=mean, scalar2=var, op0=ALU.subtract, op1=ALU.mult
    )
    nc.vector.tensor_tensor(out, out, g_row, ALU.mult)
    nc.vector.tensor_tensor(out, out, b_row, ALU.add)
    return out


def build_kernel():
    import concourse.tile_utils as tile_utils

    tile_utils.max_sbuf_usage = 208 * 1024

    nc = bacc.Bacc()

    # ---- DRAM I/O ----
    xT = nc.dram_tensor("xT", [H, T], f32, kind="ExternalInput")
    ent = nc.dram_tensor("ent", [NE, H], f32, kind="ExternalInput")
    entT = nc.dram_tensor("entT", [H, NE], f32, kind="ExternalInput")
    ee_w1 = nc.dram_tensor("ee_w1", [H, H2], f32, kind="ExternalInput")
    ee_b1 = nc.dram_tensor("ee_b1", [1, H2], f32, kind="ExternalInput")
    ee_w2 = nc.dram_tensor("ee_w2", [H2, H], f32, kind="ExternalInput")
    ee_b2 = nc.dram_tensor("ee_b2", [1, H], f32, kind="ExternalInput")
    ee_g = nc.dram_tensor("ee_g", [1, H], f32, kind="ExternalInput")
    ee_beta = nc.dram_tensor("ee_beta", [1, H], f32, kind="ExternalInput")
    re_w1 = nc.dram_tensor("re_w1", [H, H], f32, kind="ExternalInput")
    re_b1 = nc.dram_tensor("re_b1", [1, H], f32, kind="ExternalInput")
    re_w2 = nc.dram_tensor("re_w2", [H, H], f32, kind="ExternalInput")
    re_b2 = nc.dram_tensor("re_b2", [1, H], f32, kind="ExternalInput")
    re_g = nc.dram_tensor("re_g", [1, H], f32, kind="ExternalInput")
    re_beta = nc.dram_tensor("re_beta", [1, H], f32, kind="ExternalInput")
    rn_w1 = nc.dram_tensor("rn_w1", [3 * H, H2], f32, kind="ExternalInput")
    rn_b1 = nc.dram_tensor("rn_b1", [1, H2], f32, kind="ExternalInput")
    rn_w2 = nc.dram_tensor("rn_w2", [H2, H], f32, kind="ExternalInput")
    rn_b2 = nc.dram_tensor("rn_b2", [1, H], f32, kind="ExternalInput")
    rn_g = nc.dram_tensor("rn_g", [1, H], f32, kind="ExternalInput")
    rn_beta = nc.dram_tensor("rn_beta", [1, H], f32, kind="ExternalInput")
    vn_w1 = nc.dram_tensor("vn_w1", [H, H // 2], f32, kind="ExternalInput")
    vn_b1 = nc.dram_tensor("vn_b1", [1, H // 2], f32, kind="ExternalInput")
    vn_w2 = nc.dram_tensor("vn_w2", [H // 2, 1], f32, kind="ExternalInput")
    vn_b2 = nc.dram_tensor("vn_b2", [1, 1], f32, kind="ExternalInput")

    ef_out = nc.dram_tensor("ef_out", [T, H], f32, kind="ExternalOutput")
    rf_out = nc.dram_tensor("rf_out", [1, H], f32, kind="ExternalOutput")
    ret_out = nc.dram_tensor("ret_out", [TOPK, H], f32, kind="ExternalOutput")
    sims_out = nc.dram_tensor("sims_out", [1, NE], f32, kind="ExternalOutput")
    idx_out = nc.dram_tensor("idx_out", [1, TOPK], i32, kind="ExternalOutput")
    ro_out = nc.dram_tensor("ro_out", [1, H], f32, kind="ExternalOutput")
    vs_out = nc.dram_tensor("vs_out", [1, 1], f32, kind="ExternalOutput")

    from contextlib import ExitStack

    with tile.TileContext(nc) as tc, ExitStack() as stack:
        # ---------- persistent pools ----------
        cpool = stack.enter_context(tc.tile_pool(name="consts", bufs=1))
        colp = stack.enter_context(tc.tile_pool(name="colp", bufs=1))
        rows = None
        scal_p = None

        ident = cpool.tile([P, P], f32)
        make_identity(nc, ident)
        b1_col = cpool.tile([P, MO1], f32)
        nc.sync.dma_start(out=b1_col, in_=ee_b1.rearrange("a (m p) -> p (a m)", p=P))
        b2_bc = cpool.tile([P, H], f32)
        nc.sync.dma_start(out=b2_bc, in_=ee_b2[:, :].to_broadcast([P, H]))
        g_bc = cpool.tile([P, H], f32)
        nc.sync.dma_start(out=g_bc, in_=ee_g[:, :].to_broadcast([P, H]))
        beta_bc = cpool.tile([P, H], f32)
        nc.sync.dma_start(out=beta_bc, in_=ee_beta[:, :].to_broadcast([P, H]))
        eps_t = cpool.tile([P, 1], f32)
        nc.vector.memset(eps_t, EPS)
        eps1 = cpool.tile([1, 1], f32)
        nc.vector.memset(eps1, EPS)
        ones_row = cpool.tile([1, P], f32)
        nc.vector.memset(ones_row, 1.0)
        one1 = cpool.tile([1, 1], f32)
        nc.vector.memset(one1, 1.0)
        pooled_acc = cpool.tile([P, KO1], f32)
        nc.vector.memset(pooled_acc, 0.0)
        rin_col = colp.tile([P, 24], f32r, name="rin_col", tag="rin_col")

        def row_tile(n, nm):
            if n > 1024:
                return rows.tile([1, n], f32, name=nm, tag="rows2", bufs=2)
            return rows.tile([1, n], f32, name=nm, tag="rows", bufs=6)

        def load_row(dram, n, nm):
            t_ = row_tile(n, nm)
            nc.sync.dma_start(out=t_, in_=dram[:, :])
            return t_

        # ---------- fused phase A + interleaved vector chain ----------
        with (
            tc.tile_pool(name="w1p", bufs=1) as w1p,
            tc.tile_pool(name="w2p", bufs=1) as w2p,
            tc.tile_pool(name="xtp", bufs=1) as xtp,
            tc.tile_pool(name="htp", bufs=1) as htp,
            tc.tile_pool(name="efp", bufs=4) as efp,
            tc.tile_pool(name="lnp", bufs=3) as lnp,
            tc.tile_pool(name="psmm", bufs=8, space="PSUM") as psmm,
        ):
            NTT = TB // P

            # block-0 activations first so the PE can start ASAP
            xt0 = xtp.tile([P, KO1, TB], f32r, name="xt", tag="xt")
            nc.sync.dma_start(
                out=xt0,
                in_=xT[:, 0:TB].rearrange("(ko p) t -> p ko t", p=P).bitcast(f32r),
            )

            # --- W1 + W2 resident as f32r ---
            w1r = []
            for ko in range(KO1):
                wr = w1p.tile([P, H2], f32r, name=f"w1r{ko}", tag=f"w1r{ko}")
                nc.sync.dma_start(
                    out=wr, in_=ee_w1[ko * P : (ko + 1) * P, :].bitcast(f32r)
                )
                w1r.append(wr)
            w2r = []
            for ko in range(KO2):
                wr = w2p.tile([P, H], f32r, name=f"w2r{ko}", tag=f"w2r{ko}")
                nc.sync.dma_start(
                    out=wr, in_=ee_w2[ko * P : (ko + 1) * P, :].bitcast(f32r)
                )
                w2r.append(wr)

            # --- chain segment builders (emitted between blocks) ---
            def vec_mlp_layer(w_dram, in_col, n_out, bias_row, nm):
                """[1, n_out] = in_col.T @ W + bias, f32r, weights streamed."""
                out_row = row_tile(n_out, f"{nm}_row")
                nko = in_col.shape[1]
                for n in range((n_out + 511) // 512):
                    nsz = min(512, n_out - n * 512)
                    ps_a = psva.tile([1, 512], f32, name=f"{nm}ps{n}", tag="vecacc")
                    for ko in range(nko):
                        wch = rewp.tile([P, 512], f32r, name=f"{nm}w", tag="rew")
                        nc.sync.dma_start(
                            out=wch[:, :nsz],
                            in_=w_dram[
                                ko * P : (ko + 1) * P, n * 512 : n * 512 + nsz
                            ].bitcast(f32r),
                        )
                        nc.tensor.matmul(
                            ps_a[:, :nsz],
                            lhsT=in_col[:, ko : ko + 1],
                            rhs=wch[:, :nsz],
                            start=(ko == 0),
                            stop=(ko == nko - 1),
                        )
                    nc.vector.tensor_tensor(
                        out_row[:, n * 512 : n * 512 + nsz],
                        ps_a[:, :nsz],
                        bias_row[:, n * 512 : n * 512 + nsz],
                        ALU.add,
                    )
                return out_row

            def row_to_col(row, n_chunks, nm, dtype=f32):
                ps_col = psca.tile([P, 16], f32, name=f"{nm}_ps", tag="colacc")
                for ko in range(n_chunks):
                    nc.tensor.matmul(
                        ps_col[:, ko : ko + 1],
                        lhsT=row[:, ko * P : (ko + 1) * P],
                        rhs=one1,
                        start=True,
                        stop=True,
                    )
                col = colp.tile([P, n_chunks], dtype, name=f"{nm}_col", tag=f"{nm}_col")
                nc.vector.tensor_copy(col, ps_col[:, :n_chunks])
                return col

            state = {}

            def seg_re1():
                reb1 = load_row(re_b1, H, "reb1")
                h1 = vec_mlp_layer(re_w1, pooled_col, H, reb1, "re1")
                nc.scalar.activation(h1, h1, AF.Gelu)
                state["g1_col"] = row_to_col(h1, KO1, "g1", dtype=f32r)

            def seg_re2():
                reb2 = load_row(re_b2, H, "reb2")
                r_row = vec_mlp_layer(re_w2, state["g1_col"], H, reb2, "re2")
                reg_row = load_row(re_g, H, "reg")
                rebeta_row = load_row(re_beta, H, "rebeta")
                rf_row = _row_layernorm(
                    nc, rows, scal_p, r_row, reg_row, rebeta_row, eps1, "rf"
                )
                nc.sync.dma_start(out=rf_out[:, :], in_=rf_row)
                state["rf_col"] = row_to_col(rf_row, KO1, "rf", dtype=f32r)
                nc.vector.tensor_copy(rin_col[:, 8:16], state["rf_col"])

            def seg_sims():
                rf_col = state["rf_col"]
                sims_row = row_tile(NE, "sims_row")
                for n in range(2):
                    nsz = 512 if n == 0 else NE - 512
                    ps_s = psva.tile([1, 512], f32, name=f"simsps{n}", tag="vecacc")
                    for ko in range(KO1):
                        ech = etsp.tile([P, 512], f32r, name="ech", tag="ets")
                        nc.sync.dma_start(
                            out=ech[:, :nsz],
                            in_=entT[
                                ko * P : (ko + 1) * P, n * 512 : n * 512 + nsz
                            ].bitcast(f32r),
                        )
                        nc.tensor.matmul(
                            ps_s[:, :nsz],
                            lhsT=rf_col[:, ko : ko + 1],
                            rhs=ech[:, :nsz],
                            start=(ko == 0),
                            stop=(ko == KO1 - 1),
                        )
                    nc.vector.tensor_copy(
                        sims_row[:, n * 512 : n * 512 + nsz], ps_s[:, :nsz]
                    )
                nc.sync.dma_start(out=sims_out[:, :], in_=sims_row)
                mxv = colp.tile([1, 8], f32, name="mxv", tag="mxv")
                mxi = colp.tile([1, 8], u32, name="mxi", tag="mxi")
                nc.vector.max_with_indices(mxv, mxi, sims_row)
                mii = colp.tile([1, 8], i32, name="mii", tag="mii")
                nc.vector.tensor_copy(mii, mxi)
                nc.sync.dma_start(out=idx_out[:, :], in_=mii[:, :TOPK])
                mif = colp.tile([1, 8], f32, name="mif", tag="mif")
                nc.vector.tensor_copy(mif, mxi)
                ps_ib = psca.tile([P, 8], f32, name="ps_ib", tag="colacc")
                nc.tensor.matmul(
                    ps_ib, lhsT=ones_row, rhs=mif, start=True, stop=True
                )
                idx_b = colp.tile([P, 8], f32, name="idx_b", tag="idx_b")
                nc.vector.tensor_copy(idx_b, ps_ib)
                iota_i = colp.tile([P, 8], i32, name="iota_i", tag="iota_i")
                nc.gpsimd.iota(
                    iota_i, pattern=[[P, 8]], base=0, channel_multiplier=1
                )
                iota_f = colp.tile([P, 8], f32, name="iota_f", tag="iota_f")
                nc.vector.tensor_copy(iota_f, iota_i)
                onehot = colp.tile([P, 8, TOPK], f32, name="onehot", tag="onehot")
                for et in range(8):
                    nc.vector.tensor_tensor(
                        onehot[:, et, :],
                        idx_b[:, :TOPK],
                        iota_f[:, et : et + 1].to_broadcast([P, TOPK]),
                        ALU.is_equal,
                    )
                state["onehot"] = onehot

            segments = [seg_re1, seg_re2, seg_sims]

            # --- phase A blocks with interleaved segments ---
            pooled_col = colp.tile([P, KO1], f32r, name="pooled_col", tag="pooled_col")
            for b in range(NBLK):
                if b == 0:
                    xt = xt0
                else:
                    xt = xtp.tile([P, KO1, TB], f32r, name="xt", tag="xt")
                    nc.sync.dma_start(
                        out=xt,
                        in_=xT[:, b * TB : (b + 1) * TB]
                        .rearrange("(ko p) t -> p ko t", p=P)
                        .bitcast(f32r),
                    )
                # pooled: exact f32 view, 2-stage reduce for short sum chains
                red2 = lnp.tile([P, KO1, 8], f32, name="red2", tag="red2")
                nc.vector.tensor_reduce(
                    red2,
                    xt.bitcast(f32).rearrange("p ko (c t) -> p ko c t", c=8),
                    axis=mybir.AxisListType.X,
                    op=ALU.add,
                )
                red = lnp.tile([P, KO1], f32, name="red", tag="red")
                nc.vector.tensor_reduce(
                    red, red2, axis=mybir.AxisListType.X, op=ALU.add
                )
                nc.vector.tensor_tensor(pooled_acc, pooled_acc, red, ALU.add)
                if b == NBLK - 1:
                    nc.scalar.mul(pooled_col, pooled_acc, 1.0 / T)
                    nc.vector.tensor_copy(rin_col[:, 16:24], pooled_col)

                ht = htp.tile([P, MO1, TB], f32r, name="ht", tag="ht")
                for m in range(MO1):
                    ps1 = psmm.tile([P, TB], f32, name="ps1", tag="mm")
                    for ko in range(KO1):
                        nc.tensor.matmul(
                            ps1,
                            lhsT=w1r[ko][:, m * P : (m + 1) * P],
                            rhs=xt[:, ko, :],
                            start=(ko == 0),
                            stop=(ko == KO1 - 1),
                        )
                    nc.scalar.activation(
                        ht[:, m, :], ps1, AF.Gelu, bias=b1_col[:, m : m + 1], scale=1.0
                    )

                ef_blk = [
                    efp.tile([P, H], f32, name=f"ef{tt}", tag="ef")
                    for tt in range(NTT)
                ]
                for tt in range(NTT):
                    for g in range(2):
                        acc = psmm.tile([P, 512], f32, name="acc", tag="mm")
                        for ko in range(KO2):
                            nc.tensor.matmul(
                                acc,
                                lhsT=ht[:, ko, tt * P : (tt + 1) * P],
                                rhs=w2r[ko][:, g * 512 : (g + 1) * 512],
                                start=(ko == 0),
                                stop=(ko == KO2 - 1),
                            )
                        nc.vector.tensor_tensor(
                            ef_blk[tt][:, g * 512 : (g + 1) * 512],
                            acc,
                            b2_bc[:, g * 512 : (g + 1) * 512],
                            ALU.add,
                        )

                for tt in range(NTT):
                    ef_t = ef_blk[tt]
                    stats = lnp.tile([P, 2, 6], f32, name="stats", tag="stats")
                    for sg in range(2):
                        nc.vector.bn_stats(
                            stats[:, sg, :], ef_t[:, sg * 512 : (sg + 1) * 512]
                        )
                    mv = lnp.tile([P, 2], f32, name="mv", tag="mv")
                    nc.vector.bn_aggr(mv, stats)
                    nc.scalar.activation(
                        mv[:, 1:2], mv[:, 1:2], AF.Sqrt, bias=eps_t, scale=1.0
                    )
                    nc.vector.reciprocal(mv[:, 1:2], mv[:, 1:2])
                    nc.vector.tensor_scalar(
                        ef_t,
                        ef_t,
                        scalar1=mv[:, 0:1],
                        scalar2=mv[:, 1:2],
                        op0=ALU.subtract,
                        op1=ALU.mult,
                    )
                    nc.vector.tensor_tensor(ef_t, ef_t, g_bc, ALU.mult)
                    nc.vector.tensor_tensor(ef_t, ef_t, beta_bc, ALU.add)
                    nc.sync.dma_start(
                        out=ef_out[b * TB + tt * P : b * TB + (tt + 1) * P, :],
                        in_=ef_t,
                    )


        # ---------- post: reasoning + validation ----------
        with (
            tc.tile_pool(name="rows", bufs=3) as rows,
            tc.tile_pool(name="scal", bufs=12) as scal_p,
            tc.tile_pool(name="entp", bufs=1) as entp,
            tc.tile_pool(name="rewp2", bufs=3) as rewp2,
            tc.tile_pool(name="etsp2", bufs=3) as etsp2,
            tc.tile_pool(name="rnwp", bufs=6) as rnwp,
            tc.tile_pool(name="vnwp", bufs=2) as vnwp,
            tc.tile_pool(name="psb_va", bufs=6, space="PSUM") as psb_va,
            tc.tile_pool(name="psb_ca", bufs=2, space="PSUM") as psb_ca,
        ):
            # run the relation/sims chain with post-A pools
            rewp = rewp2
            etsp = etsp2
            psva = psb_va
            psca = psb_ca
            seg_re1()
            seg_re2()
            seg_sims()

            # --- gather retrieved entities (resident table, exact fp32) ---
            onehot = state["onehot"]
            ent_nat = []
            for et in range(8):
                rows_e = P if et < 7 else NE - 7 * P
                t_ = entp.tile([P, H], f32, name=f"ent{et}", tag=f"ent{et}")
                nc.sync.dma_start(
                    out=t_[:rows_e, :], in_=ent[et * P : et * P + rows_e, :]
                )
                ent_nat.append((t_, rows_e))
            ret_col = colp.tile([P, KO1, TOPK], f32, name="ret_col", tag="ret_col")
            for fo in range(KO1):
                ps_g = psb_ca.tile([P, TOPK], f32, name="ps_g", tag="colacc")
                for et in range(8):
                    t_, rows_e = ent_nat[et]
                    nc.tensor.matmul(
                        ps_g,
                        lhsT=t_[:rows_e, fo * P : (fo + 1) * P],
                        rhs=onehot[:rows_e, et, :],
                        start=(et == 0),
                        stop=(et == 7),
                    )
                nc.vector.tensor_copy(ret_col[:, fo, :], ps_g)
            ret_row = rows.tile([TOPK, H], f32, name="ret_row", tag="rows", bufs=6)
            for fo in range(KO1):
                ps_r5 = psb_ca.tile([TOPK, P], f32, name="ps_r5", tag="colacc")
                nc.tensor.transpose(ps_r5, ret_col[:, fo, :], ident)
                nc.vector.tensor_copy(ret_row[:, fo * P : (fo + 1) * P], ps_r5)
            nc.sync.dma_start(out=ret_out[:, :], in_=ret_row)
            ev_col = colp.tile([P, KO1], f32, name="ev_col", tag="ev_col")
            nc.vector.tensor_reduce(
                ev_col, ret_col, axis=mybir.AxisListType.X, op=ALU.add
            )
            nc.scalar.mul(ev_col, ev_col, 1.0 / TOPK)
            nc.vector.tensor_copy(rin_col[:, 0:8], ev_col)

            def row_to_col2(row, n_chunks, nm):
                ps_col = psb_ca.tile([P, 16], f32, name=f"{nm}_ps", tag="colacc")
                for ko in range(n_chunks):
                    nc.tensor.matmul(
                        ps_col[:, ko : ko + 1],
                        lhsT=row[:, ko * P : (ko + 1) * P],
                        rhs=one1,
                        start=True,
                        stop=True,
                    )
                col = colp.tile([P, n_chunks], f32, name=f"{nm}_col", tag=f"{nm}_col")
                nc.vector.tensor_copy(col, ps_col[:, :n_chunks])
                return col

            rnb1 = load_row(rn_b1, H2, "rnb1")
            h1r_row = row_tile(H2, "h1r_row")
            ps_rn = [
                psb_va.tile([1, 512], f32, name=f"ps_rn{n}", tag="vecacc")
                for n in range(4)
            ]
            rn1_order = list(range(16, 24)) + list(range(8, 16)) + list(range(8))
            for i, ko in enumerate(rn1_order):
                rwr = rnwp.tile([P, H2], f32r, name="rnw", tag="rnw")
                nc.sync.dma_start(
                    out=rwr, in_=rn_w1[ko * P : (ko + 1) * P, :].bitcast(f32r)
                )
                for n in range(4):
                    nc.tensor.matmul(
                        ps_rn[n],
                        lhsT=rin_col[:, ko : ko + 1],
                        rhs=rwr[:, n * 512 : (n + 1) * 512],
                        start=(i == 0),
                        stop=(i == 23),
                    )
            for n in range(4):
                nc.vector.tensor_tensor(
                    h1r_row[:, n * 512 : (n + 1) * 512],
                    ps_rn[n],
                    rnb1[:, n * 512 : (n + 1) * 512],
                    ALU.add,
                )
            nc.scalar.activation(h1r_row, h1r_row, AF.Gelu)
            g1r_col = row_to_col2(h1r_row, 16, "g1r")
            g1r_r = colp.tile([P, 16], f32r, name="g1r_r", tag="g1r_r")
            nc.vector.tensor_copy(g1r_r, g1r_col)

            rnb2 = load_row(rn_b2, H, "rnb2")
            r2_row = row_tile(H, "r2_row")
            ps_rn2 = [
                psb_va.tile([1, 512], f32, name=f"ps_rn2{n}", tag="vecacc")
                for n in range(2)
            ]
            for ko in range(16):
                rwr = rnwp.tile([P, H], f32r, name="rnw2", tag="rnw")
                nc.sync.dma_start(
                    out=rwr, in_=rn_w2[ko * P : (ko + 1) * P, :].bitcast(f32r)
                )
                for n in range(2):
                    nc.tensor.matmul(
                        ps_rn2[n],
                        lhsT=g1r_r[:, ko : ko + 1],
                        rhs=rwr[:, n * 512 : (n + 1) * 512],
                        start=(ko == 0),
                        stop=(ko == 15),
                    )
            for n in range(2):
                nc.vector.tensor_tensor(
                    r2_row[:, n * 512 : (n + 1) * 512],
                    ps_rn2[n],
                    rnb2[:, n * 512 : (n + 1) * 512],
                    ALU.add,
                )
            rng_row = load_row(rn_g, H, "rng_row")
            rnbeta_row = load_row(rn_beta, H, "rnbeta_row")
            ro_row = _row_layernorm(
                nc, rows, scal_p, r2_row, rng_row, rnbeta_row, eps1, "ro"
            )
            nc.sync.dma_start(out=ro_out[:, :], in_=ro_row)

            ro_col = row_to_col2(ro_row, KO1, "roc")
            vnb1_row = load_row(vn_b1, H // 2, "vnb1_row")
            ps_v = psb_va.tile([1, 512], f32, name="ps_v", tag="vecacc")
            for ko in range(KO1):
                vw = vnwp.tile([P, 512], f32, name="vnw", tag="vnw")
                nc.sync.dma_start(out=vw, in_=vn_w1[ko * P : (ko + 1) * P, :])
                nc.tensor.matmul(
                    ps_v,
                    lhsT=ro_col[:, ko : ko + 1],
                    rhs=vw,
                    start=(ko == 0),
                    stop=(ko == KO1 - 1),
                )
            v1_row = row_tile(512, "v1_row")
            nc.vector.tensor_tensor(v1_row, ps_v, vnb1_row, ALU.add)
            nc.scalar.activation(v1_row, v1_row, AF.Gelu)
            v1_col = row_to_col2(v1_row, 4, "v1")
            vnw2 = colp.tile([P, 4, 1], f32, name="vnw2", tag="vnw2")
            nc.sync.dma_start(
                out=vnw2, in_=vn_w2.rearrange("(ko p) n -> p ko n", p=P)
            )
            vnb2_row = scal_p.tile([1, 1], f32, name="vnb2_row", tag="scal")
            nc.sync.dma_start(out=vnb2_row, in_=vn_b2[:, :])
            ps_v2 = psb_va.tile([1, 1], f32, name="ps_v2", tag="vecacc")
            for ko in range(4):
                nc.tensor.matmul(
                    ps_v2,
                    lhsT=v1_col[:, ko : ko + 1],
                    rhs=vnw2[:, ko, :],
                    start=(ko == 0),
                    stop=(ko == 3),
                )
            v2_row = scal_p.tile([1, 1], f32, name="v2_row", tag="scal")
            nc.vector.tensor_tensor(v2_row, ps_v2, vnb2_row, ALU.add)
            nc.scalar.activation(v2_row, v2_row, AF.Sigmoid)
            nc.sync.dma_start(out=vs_out[:, :], in_=v2_row)

    nc.compile()
    return nc


_NC_CACHE = None


def _get_nc():
    global _NC_CACHE
    if _NC_CACHE is None:
        _NC_CACHE = build_kernel()
    return _NC_CACHE


def run(inputs, trace=False):
    from concourse.bass_utils import run_bass_kernel_spmd

    nc = _get_nc()
    hs = np.ascontiguousarray(inputs["hidden_states"], dtype=np.float32)
    B = hs.shape[0]
    assert B == N_CORES

    def row(name, n):
        return np.ascontiguousarray(
            np.asarray(inputs[name], dtype=np.float32).reshape(1, n)
        )

    E = np.ascontiguousarray(inputs["entity_embeddings"], np.float32)
    shared = {
        "ent": E,
        "entT": np.ascontiguousarray(E.T),
        "ee_w1": np.ascontiguousarray(inputs["ee_w1"], np.float32),
        "ee_b1": row("ee_b1", H2),
        "ee_w2": np.ascontiguousarray(inputs["ee_w2"], np.float32),
        "ee_b2": row("ee_b2", H),
        "ee_g": row("ee_g", H),
        "ee_beta": row("ee_beta", H),
        "re_w1": np.ascontiguousarray(inputs["re_w1"], np.float32),
        "re_b1": row("re_b1", H),
        "re_w2": np.ascontiguousarray(inputs["re_w2"], np.float32),
        "re_b2": row("re_b2", H),
        "re_g": row("re_g", H),
        "re_beta": row("re_beta", H),
        "rn_w1": np.ascontiguousarray(inputs["rn_w1"], np.float32),
        "rn_b1": row("rn_b1", H2),
        "rn_w2": np.ascontiguousarray(inputs["rn_w2"], np.float32),
        "rn_b2": row("rn_b2", H),
        "rn_g": row("rn_g", H),
        "rn_beta": row("rn_beta", H),
        "vn_w1": np.ascontiguousarray(inputs["vn_w1"], np.float32),
        "vn_b1": row("vn_b1", H // 2),
        "vn_w2": np.ascontiguousarray(inputs["vn_w2"], np.float32),
        "vn_b2": row("vn_b2", 1),
    }
    in_maps = [
        dict(shared, xT=np.ascontiguousarray(hs[c].T)) for c in range(B)
    ]
    res = run_bass_kernel_spmd(
        nc, in_maps, core_ids=list(range(N_CORES)), trace=trace
    )
    r = res.results
    ef = np.stack([r[c]["ef_out"] for c in range(B)])
    rf = np.stack([r[c]["rf_out"][0] for c in range(B)])
    ret = np.stack([r[c]["ret_out"] for c in range(B)])
    sims = np.stack([r[c]["sims_out"][0] for c in range(B)])
    idx = np.stack([r[c]["idx_out"][0] for c in range(B)]).astype(np.int32)
    ro = np.stack([r[c]["ro_out"][0] for c in range(B)])
    vs = np.stack([r[c]["vs_out"][0] for c in range(B)])
    out = (ef, rf, ret, sims, idx, ro, vs)
    return (out, res) if trace else out


def kernel(**inputs):
    return run(inputs, trace=False)
